# revision 26
# baseline (speedup 1.0000x reference)
"""MiniCausalAttention on 8 NeuronCores (Trainium2, Bass/Tile).

Problem: x[4,2048,1024] fp32; q/k/v = x@w+b; causal softmax(q k^T/sqrt(D)) @ v.

Sharding: 8 cores = (batch b in 0..3) x (half h in 0..1). Core (b,h) handles
query tiles t' = 2t+h for t in 0..7 (interleaved 128-row tiles), so every
core sees the SAME set of causal key-extents nk(t) = 256*(t+1) -> one SPMD
program, perfectly balanced.

Projection reassociation (exact algebra, host-precomputed M = Wq Wk^T and
u = Wk bq):
  scores  S = q k^T = x_q M x^T + 1 (x) (x u)^T  (+ per-query terms that
          cancel in softmax and are dropped)
  output  O = P_norm v = [(P x) Wv] / rowsum + bv
so neither K nor V is ever materialized. The key-bias u·x_k term is folded
into G^T = (x_q M)^T at PSUM->SBUF copy time as a per-partition activation
bias (G'^T[d,q] = G^T[d,q] + u[d]); 1/rowsum is folded into the Z copy; the
output bias bv is folded into the O accumulation as a rank-1 K=1 matmul.

Precision: G is computed in bf16 (scores accuracy), then stored fp8-e4m3;
x^T is shipped fp8 -> the S matmul runs fp8 DoubleRow (K=256/instruction,
1 col/cycle = 2x bf16 FLOP rate) costing ~1.3e-2 rel err total (tol 2e-2,
validated in numpy sim; fp8 anywhere in the P*V path fails tolerance, and
fp8 multi-plane residual tricks lose to bf16 since DR is 1 cyc/col).
P/Z/O stay bf16; PSUM accumulation fp32; softmax statistics fp32.

Scheduling (PE is the bottleneck; it streams 1 col/cycle at 2.0-2.4GHz with
LDWEIGHTS fully hidden, so only column count and stalls matter):
- Inputs stream in first-use order on two DMA queues; the m/xtq weights for
  the score projection G lead so the ct-outer G loop (8 PSUM accumulators,
  dt-halves with scalar/vector-alternated copies) starts ~1.5us after DMA.
- Tiles 0-3 of phase B run right after the qc=0 half of G (they only need
  the first x chunks), filling the input-DMA window with PE work; their
  ZO/EP stages are deferred. The qc=1 half of G then runs on 2 spare PSUM
  banks, and tiles 4-7 drain the deferred ZO stages two per tile.
- Within a tile the PE issue order software-pipelines:
    S(t,0) S(t,1) ZO(prev) EP(prev) TZ(t,0) S(t,2) TZ(t,1) ... ZC(t)
  so the exp latency (scalar), the z-copy, and the o-copy all hide under
  score/Z matmuls. Tile 7 processes its masked chunk third so the closing
  rowsum chain stays off the critical path, and its EP splits across the
  scalar+vector engines with per-half DMAs.
"""

import sys

if "/opt/trn_rl_repo" not in sys.path:
    sys.path.insert(0, "/opt/trn_rl_repo")

import numpy as np
import ml_dtypes

import concourse.bass as bass  # noqa: F401
import concourse.tile as tile
from concourse import bacc, mybir
from concourse.bass_utils import run_bass_kernel_spmd
from concourse.masks import make_identity

BF16 = mybir.dt.bfloat16
F32 = mybir.dt.float32
FP8 = mybir.dt.float8e4
AF = mybir.ActivationFunctionType
DR = mybir.MatmulPerfMode.DoubleRow

B, L, D = 4, 2048, 1024
P = 128
NQT = 8          # q-tiles per core, 128 rows each
SCALE = 1.0 / 32.0   # 1/sqrt(D)
NEG = -1.0e30

_CACHED = {}


def build_nc():
    nc = bacc.Bacc(None, target_bir_lowering=False)

    xt = nc.declare_dram_parameter("xt", [D, L], FP8, isOutput=False)      # x^T fp8
    xr = nc.declare_dram_parameter("xr", [L, D], BF16, isOutput=False)     # x rows
    xtq = nc.declare_dram_parameter("xtq", [D, D], BF16, isOutput=False)   # q cols of x^T
    mm_w = nc.declare_dram_parameter("mm_w", [D, D], BF16, isOutput=False)  # Wq Wk^T
    wv = nc.declare_dram_parameter("wv", [D, D], BF16, isOutput=False)
    um = nc.declare_dram_parameter("um", [P, 8], F32, isOutput=False)      # Wk bq
    bvr = nc.declare_dram_parameter("bvr", [1, D], BF16, isOutput=False)
    mask = nc.declare_dram_parameter("mask", [P, 256], F32, isOutput=False)
    out = nc.declare_dram_parameter("out", [D, D], F32, isOutput=True)

    with tile.TileContext(nc) as tc:
        with tc.tile_pool(name="persist", bufs=1) as persist:
            xt_sb = persist.tile([P, 8, L], FP8)     # x^T: [d-part, ct, token]
            xr_sb = persist.tile([P, 16, D], BF16)   # x: [tok-part, tt, d]
            gt_sb = persist.tile([P, 8, D], FP8)     # G'^T: [d-part, dt, qcol]
            xtq_sb = persist.tile([P, 8, D], BF16)
            m_sb = persist.tile([P, 8, D], BF16)
            wv_sb = persist.tile([P, 8, D], BF16)
            um_sb = persist.tile([P, 8], F32)
            bvr_sb = persist.tile([1, D], BF16)
            mask_sb = persist.tile([P, 256], F32)
            ident = persist.tile([P, P], BF16)
            ones_sb = persist.tile([1, P], BF16)
            warm_sb = persist.tile([P, 4], F32)      # warmup matmul sink

            make_identity(nc, ident)
            nc.vector.memset(ones_sb, 1.0)

            # Input streams, ordered by first use. The sync queue carries the
            # critical-path stream in exact need-order (grouped transfers to
            # stay bandwidth- not issue-bound): m/xtq-h1 for G(qc0), then the
            # x chunks tiles 0-3 consume, then xtq-h2 for G(qc1). The gpsimd
            # queue carries the late-needed bulk (wv for ZO, xt-c1/xr-hi for
            # tiles 4-7).
            for i in range(2):
                nc.sync.dma_start(out=m_sb[:, i, :],
                                  in_=mm_w[i * P:(i + 1) * P, :])
            for i in range(4):
                nc.sync.dma_start(out=xtq_sb[:, i, :512],
                                  in_=xtq[i * P:(i + 1) * P, :512])
            for i in range(2, 6):
                nc.sync.dma_start(out=m_sb[:, i, :],
                                  in_=mm_w[i * P:(i + 1) * P, :])
            for i in range(4, 8):
                nc.sync.dma_start(out=xtq_sb[:, i, :512],
                                  in_=xtq[i * P:(i + 1) * P, :512])
            for i in range(6, 8):
                nc.sync.dma_start(out=m_sb[:, i, :],
                                  in_=mm_w[i * P:(i + 1) * P, :])
            nc.sync.dma_start(out=um_sb, in_=um[:, :])
            nc.sync.dma_start(out=bvr_sb, in_=bvr[:, :])
            nc.sync.dma_start(out=mask_sb, in_=mask[:, :])
            for i in range(8):
                nc.sync.dma_start(out=xtq_sb[:, i, 512:],
                                  in_=xtq[i * P:(i + 1) * P, 512:])
            for i in range(8):
                nc.gpsimd.dma_start(out=xt_sb[:, i, :1024],
                                    in_=xt[i * P:(i + 1) * P, :1024])
            for tt in range(8):
                nc.gpsimd.dma_start(out=xr_sb[:, tt, :],
                                    in_=xr[tt * P:(tt + 1) * P, :])
            for i in range(8):
                nc.gpsimd.dma_start(out=wv_sb[:, i, :],
                                    in_=wv[i * P:(i + 1) * P, :])
            for i in range(8):
                nc.gpsimd.dma_start(out=xt_sb[:, i, 1024:],
                                    in_=xt[i * P:(i + 1) * P, 1024:])
            for tt in range(8, 16):
                nc.gpsimd.dma_start(out=xr_sb[:, tt, :],
                                    in_=xr[tt * P:(tt + 1) * P, :])

            # ---------- Phase A/B interleaved schedule -------------------
            # G = (x_q M)^T + u, computed ct-outer over PSUM accumulators so
            # the first matmul needs only the first m/xtq DMA. The qc=0
            # column-half runs first on all 8 banks; then tiles 0-1 of the
            # attention phase run (they only need qc=0 scores + the first x
            # chunks) while the rest of the inputs stream in; then the qc=1
            # half runs on 2 banks; then the remaining tiles.
            #
            # Phase-B stages per tile t (nk = 256*(t+1), nkc = #512-chunks):
            #   S(t,c): fp8 DoubleRow score matmuls into psS
            #   E(t,c): mask (last chunk) + exp -> p_sb, rowsum accum, rinv
            #   TZ(t,kt): P^T transpose + copy + 2 Z matmuls into pz
            #   ZC(t): pz -> z_sb (bf16), normalized by 1/rowsum
            #   ZO(t): Z^T transposes + O matmuls (+ bv rank-1 fold) into po
            #   EP(t): po -> o_sb (scalar), DMA out
            # PE issue order pipelines: S of the next chunk covers exp; ZO of
            # the previous tile covers the z-copy; S(t,0..1) covers ZC(t-1).
            with tc.tile_pool(name="psA", bufs=1, space="PSUM") as psA:
                # warmup chain: keeps the PE p-state ramped across the
                # initial input-DMA wait so G streams at full clock
                for rep in range(6):
                    for dc in range(8):
                        pb = psA.tile([P, 512], F32, tag=f"g{dc}",
                                      name=f"pb{dc}")
                        nc.tensor.matmul(pb[:, :P], ones_sb, ones_sb,
                                         start=True, stop=True)
                        if rep == 5 and dc < 2:
                            nc.scalar.copy(warm_sb[:, dc * 2:dc * 2 + 2],
                                           pb[:, :2])

                for dth in range(2):
                    pg = [psA.tile([P, 512], F32, tag=f"g{dt}", name=f"pg{dt}")
                          for dt in range(dth * 4, dth * 4 + 4)]
                    for ct in range(8):
                        for di, dt in enumerate(range(dth * 4, dth * 4 + 4)):
                            nc.tensor.matmul(
                                pg[di],
                                m_sb[:, ct, dt * P:(dt + 1) * P],
                                xtq_sb[:, ct, :512],
                                start=(ct == 0),
                                stop=(ct == 7),
                            )
                    for di, dt in enumerate(range(dth * 4, dth * 4 + 4)):
                        # alternate engines so the copy chain halves in wall
                        # time (scalar activation vs DVE tensor-scalar add)
                        if di % 2 == 0:
                            nc.scalar.activation(
                                gt_sb[:, dt, :512], pg[di],
                                AF.Identity, bias=um_sb[:, dt:dt + 1])
                        else:
                            nc.vector.tensor_scalar_add(
                                gt_sb[:, dt, :512], pg[di],
                                um_sb[:, dt:dt + 1])

            with tc.tile_pool(name="bwork", bufs=2) as bwork, \
                 tc.tile_pool(name="psB", bufs=2, space="PSUM") as psS, \
                 tc.tile_pool(name="psPZ", bufs=1, space="PSUM") as psPZ:
                psT = psS   # transposes share the psB pool (tag ptp)

                state = {}   # per-tile buffers shared across stage fns

                def tile_head(t):
                    state[(t, "p")] = bwork.tile([P, 2048], BF16, tag="p",
                                                 name="p")
                    state[(t, "rsum")] = bwork.tile([P, 4], F32, tag="rsum",
                                                    name="rsum")
                    state[(t, "pz")] = [
                        psPZ.tile([P, 512], F32, tag=f"pz{dc}", name=f"pz{dc}")
                        for dc in range(2)]

                def S_stage(t, c, finalize=None):
                    nk = 256 * (t + 1)
                    nkc = (nk + 511) // 512
                    if finalize is None:
                        finalize = (c == nkc - 1)
                    w = min(512, nk - c * 512)
                    ps = psS.tile([P, 512], F32, tag="s", name="ps")
                    for i in range(4):
                        nc.tensor.matmul(
                            ps[:, :w],
                            gt_sb[:, 2 * i:2 * i + 2, t * P:(t + 1) * P],
                            xt_sb[:, 2 * i:2 * i + 2, c * 512:c * 512 + w],
                            start=(i == 0),
                            stop=(i == 3),
                            perf_mode=DR,
                        )
                    # E stage issues immediately after (scalar/vector queues)
                    if c == nkc - 1:
                        nc.vector.tensor_add(ps[:, w - 256:w],
                                             ps[:, w - 256:w], mask_sb)
                    nc.scalar.activation(
                        state[(t, "p")][:, c * 512:c * 512 + w], ps[:, :w],
                        AF.Exp, scale=SCALE,
                        accum_out=state[(t, "rsum")][:, c:c + 1])
                    if finalize:
                        # rowsum -> 1/rowsum right after the final exp so the
                        # z-copy can normalize Z (folds the softmax divide)
                        rtot = bwork.tile([P, 1], F32, tag="rtot", name="rtot")
                        rinv = bwork.tile([P, 1], F32, tag="rinv", name="rinv")
                        nc.vector.reduce_sum(rtot, state[(t, "rsum")][:, :nkc],
                                             axis=mybir.AxisListType.X)
                        nc.vector.reciprocal(rinv, rtot)
                        state[(t, "rinv")] = rinv

                def TZ_stage(t, c, stop_kt=None):
                    nk = 256 * (t + 1)
                    if stop_kt is None:
                        stop_kt = nk // P - 1
                    w = min(512, nk - c * 512)
                    p_sb = state[(t, "p")]
                    pz = state[(t, "pz")]
                    for kt in range(c * 4, c * 4 + w // P):
                        ptp = psT.tile([P, P], BF16, tag="ptp", name="ptp")
                        nc.tensor.transpose(
                            ptp, p_sb[:, kt * P:(kt + 1) * P], ident)
                        pt_sb = bwork.tile([P, P], BF16, tag="pt", name="pt_sb")
                        nc.vector.tensor_copy(pt_sb, ptp)
                        for dc in range(2):
                            nc.tensor.matmul(
                                pz[dc],
                                pt_sb,
                                xr_sb[:, kt, dc * 512:(dc + 1) * 512],
                                start=(kt == 0),
                                stop=(kt == stop_kt),
                            )

                def ZC_stage(t):
                    # pz -> z_sb normalized by 1/rowsum (scalar and vector in
                    # parallel, one 512-chunk each). Tag per t%4: up to four
                    # z tiles are alive while their ZO stages are deferred.
                    z_sb = bwork.tile([P, D], BF16, tag=f"z{t % 4}",
                                      name="z_sb")
                    state[(t, "z")] = z_sb
                    rinv = state[(t, "rinv")]
                    nc.scalar.activation(z_sb[:, :512], state[(t, "pz")][0],
                                         AF.Copy, scale=rinv)
                    nc.vector.tensor_scalar_mul(z_sb[:, 512:],
                                                state[(t, "pz")][1], rinv)

                # ---- tiles 0-3 early: scores + Z only (ZO deferred) ------
                # They only need the qc=0 half of G and the first x chunks,
                # so they fill the input-DMA window with real PE work. The
                # qc=1 half of G runs on two spare PSUM banks, its quarter
                # passes interleaved so they cover the z-copy latencies.
                def g_qc1_quarter(psA2, q4):
                    pg = [psA2.tile([P, 512], F32, tag=f"h{j}", name=f"ph{j}")
                          for j in range(2)]
                    for ct in range(8):
                        for j in range(2):
                            dt = q4 * 2 + j
                            nc.tensor.matmul(
                                pg[j],
                                m_sb[:, ct, dt * P:(dt + 1) * P],
                                xtq_sb[:, ct, 512:],
                                start=(ct == 0),
                                stop=(ct == 7),
                            )
                    for j in range(2):
                        dt = q4 * 2 + j
                        if j == 0:
                            nc.scalar.activation(
                                gt_sb[:, dt, 512:], pg[j],
                                AF.Identity, bias=um_sb[:, dt:dt + 1])
                        else:
                            nc.vector.tensor_scalar_add(
                                gt_sb[:, dt, 512:], pg[j],
                                um_sb[:, dt:dt + 1])

                tile_head(0)
                S_stage(0, 0)
                tile_head(1)
                S_stage(1, 0)
                TZ_stage(0, 0)
                ZC_stage(0)
                tile_head(2)
                S_stage(2, 0)
                S_stage(2, 1)
                TZ_stage(1, 0)
                ZC_stage(1)
                tile_head(3)
                S_stage(3, 0)
                S_stage(3, 1)
                TZ_stage(2, 0)
                TZ_stage(2, 1)
                ZC_stage(2)
                TZ_stage(3, 0)
                TZ_stage(3, 1)
                ZC_stage(3)
                with tc.tile_pool(name="psA2", bufs=1, space="PSUM") as psA2:
                    for q4 in range(4):
                        g_qc1_quarter(psA2, q4)

                # ---- remaining tiles, with ZO/EP pipelined one tile back --
                with tc.tile_pool(name="psC", bufs=1, space="PSUM") as psC:

                    def ZO_stage(t):
                        z_sb = state[(t, "z")]
                        po = [psC.tile([P, 512], F32, tag=f"po{dc}",
                                       name=f"po{dc}")
                              for dc in range(2)]
                        state[(t, "po")] = po
                        for cc in range(8):
                            ztp = psT.tile([P, P], BF16, tag="ptp", name="ztp")
                            nc.tensor.transpose(
                                ztp, z_sb[:, cc * P:(cc + 1) * P], ident)
                            zt_sb = bwork.tile([P, P], BF16, tag="zt",
                                               name="zt_sb")
                            nc.vector.tensor_copy(zt_sb, ztp)
                            for dc in range(2):
                                nc.tensor.matmul(
                                    po[dc],
                                    zt_sb,
                                    wv_sb[:, cc, dc * 512:(dc + 1) * 512],
                                    start=(cc == 0),
                                    stop=(cc == 7),
                                )
                            if cc == 0:
                                # fold the output bias: po += 1 (x) bv (K=1)
                                for dc in range(2):
                                    nc.tensor.matmul(
                                        po[dc], ones_sb,
                                        bvr_sb[:, dc * 512:(dc + 1) * 512],
                                        start=False, stop=False)

                    def EP_stage(t, final=False):
                        # po is the finished output (normalized, biased);
                        # stage through SBUF on scalar (keeps the vector
                        # queue free for the latency-critical pt copies) and
                        # DMA out. The last tile has an idle vector queue, so
                        # split across engines with per-half DMAs instead.
                        po = state[(t, "po")]
                        o_sb = bwork.tile([P, D], F32, tag="o", name="o_sb")
                        if final:
                            nc.scalar.copy(o_sb[:, :512], po[0])
                            nc.sync.dma_start(
                                out=out[t * P:(t + 1) * P, :512],
                                in_=o_sb[:, :512])
                            nc.vector.tensor_copy(o_sb[:, 512:], po[1])
                            nc.sync.dma_start(
                                out=out[t * P:(t + 1) * P, 512:],
                                in_=o_sb[:, 512:])
                        else:
                            for dc in range(2):
                                nc.scalar.copy(
                                    o_sb[:, dc * 512:(dc + 1) * 512], po[dc])
                            nc.sync.dma_start(out=out[t * P:(t + 1) * P, :],
                                              in_=o_sb)

                    # Deferred ZO/EP stages drain two-per-tile while the
                    # remaining tiles' score/Z work keeps the PE fed.
                    pending = [0, 1, 2, 3]
                    ZO_stage(pending.pop(0))
                    EP_stage(0)
                    for t in range(4, NQT - 1):
                        nk = 256 * (t + 1)
                        nkc = (nk + 511) // 512
                        tile_head(t)
                        S_stage(t, 0)
                        S_stage(t, 1)
                        done_s = 2
                        zo = pending.pop(0)
                        ZO_stage(zo)
                        EP_stage(zo)
                        for c in range(nkc):
                            TZ_stage(t, c)
                            if done_s < nkc:
                                S_stage(t, done_s)
                                done_s += 1
                            if c == 1 and t <= 5 and pending:
                                zo = pending.pop(0)
                                ZO_stage(zo)
                                EP_stage(zo)
                        ZC_stage(t)
                        pending.append(t)
                    # tile 7 processes its masked chunk (3) third and the
                    # plain chunk 2 last, so the end-of-tile rowsum chain
                    # (mask -> exp -> reduce -> recip -> z-copy) is off the
                    # critical path; the Z accumulation stops at kt=11.
                    t = NQT - 1
                    tile_head(t)
                    S_stage(t, 0)
                    S_stage(t, 1)
                    zo = pending.pop(0)
                    ZO_stage(zo)
                    EP_stage(zo)
                    TZ_stage(t, 0, stop_kt=11)
                    S_stage(t, 3, finalize=False)
                    TZ_stage(t, 1, stop_kt=11)
                    S_stage(t, 2, finalize=True)
                    TZ_stage(t, 3, stop_kt=11)
                    TZ_stage(t, 2, stop_kt=11)
                    ZC_stage(t)
                    pending.append(t)
                    while pending:
                        zo = pending.pop(0)
                        ZO_stage(zo)
                        EP_stage(zo, final=not pending)

    nc.finalize()
    return nc


def _prep_inputs(x, wq, bq, wk, bk, wv, bv):
    bf = ml_dtypes.bfloat16
    f8 = ml_dtypes.float8_e4m3
    wq32 = np.asarray(wq, np.float32)
    wk32 = np.asarray(wk, np.float32)
    m_host = (wq32 @ wk32.T).astype(bf)                 # Wq Wk^T
    u_host = (wk32 @ np.asarray(bq, np.float32))        # Wk bq, [D]
    um = np.ascontiguousarray(u_host.reshape(8, P).T).astype(np.float32)
    wv_b = np.ascontiguousarray(wv, np.float32).astype(bf)
    bvr = np.asarray(bv, np.float32).reshape(1, D).astype(bf)

    i = np.arange(P)[:, None]
    j = np.arange(256)[None, :]
    masks = [np.where(j <= i + P * h, 0.0, NEG).astype(np.float32)
             for h in range(2)]

    in_maps = []
    for core in range(8):
        b, h = core // 2, core % 2
        xb = np.asarray(x[b], np.float32)
        xT = np.ascontiguousarray(xb.T)
        xR = xb.astype(bf)
        qcols = (np.arange(8)[:, None] * 2 + h) * P + np.arange(P)[None, :]
        xTq = np.ascontiguousarray(xT[:, qcols.ravel()]).astype(bf)
        in_maps.append({
            "xt": xT.astype(f8), "xr": xR, "xtq": xTq, "mm_w": m_host,
            "wv": wv_b, "um": um, "bvr": bvr, "mask": masks[h],
        })
    return in_maps


def kernel(x, wq, bq, wk, bk, wv, bv, _trace=False, _trace_kwargs=None):
    if "nc" not in _CACHED:
        _CACHED["nc"] = build_nc()
    nc = _CACHED["nc"]
    in_maps = _prep_inputs(x, wq, bq, wk, bk, wv, bv)
    kw = {}
    if _trace:
        kw = dict(trace=True, **(_trace_kwargs or {}))
    res = run_bass_kernel_spmd(nc, in_maps, list(range(8)), **kw)
    out = np.empty((B, L, D), np.float32)
    for core in range(8):
        b, h = core // 2, core % 2
        o = np.asarray(res.results[core]["out"], np.float32)
        out[b].reshape(16, P, D)[h::2] = o.reshape(NQT, P, D)
    if _trace:
        _CACHED["last_results"] = res
    return out


# revision 27
# speedup vs baseline: 1.0012x; 1.0012x over previous
"""MiniCausalAttention on 8 NeuronCores (Trainium2, Bass/Tile).

Problem: x[4,2048,1024] fp32; q/k/v = x@w+b; causal softmax(q k^T/sqrt(D)) @ v.

Sharding: 8 cores = (batch b in 0..3) x (half h in 0..1). Core (b,h) handles
query tiles t' = 2t+h for t in 0..7 (interleaved 128-row tiles), so every
core sees the SAME set of causal key-extents nk(t) = 256*(t+1) -> one SPMD
program, perfectly balanced.

Projection reassociation (exact algebra, host-precomputed M = Wq Wk^T and
u = Wk bq):
  scores  S = q k^T = x_q M x^T + 1 (x) (x u)^T  (+ per-query terms that
          cancel in softmax and are dropped)
  output  O = P_norm v = [(P x) Wv] / rowsum + bv
so neither K nor V is ever materialized. The key-bias u·x_k term is folded
into G^T = (x_q M)^T at PSUM->SBUF copy time as a per-partition activation
bias (G'^T[d,q] = G^T[d,q] + u[d]); 1/rowsum is folded into the Z copy; the
output bias bv is folded into the O accumulation as a rank-1 K=1 matmul.

Precision: G is computed in bf16 (scores accuracy), then stored fp8-e4m3;
x^T is shipped fp8 -> the S matmul runs fp8 DoubleRow (K=256/instruction,
1 col/cycle = 2x bf16 FLOP rate) costing ~1.3e-2 rel err total (tol 2e-2,
validated in numpy sim; fp8 anywhere in the P*V path fails tolerance, and
fp8 multi-plane residual tricks lose to bf16 since DR is 1 cyc/col).
P/Z/O stay bf16; PSUM accumulation fp32; softmax statistics fp32.

Scheduling (PE is the bottleneck; it streams 1 col/cycle at 2.0-2.4GHz with
LDWEIGHTS fully hidden, so only column count and stalls matter):
- Inputs stream in first-use order on two DMA queues; the m/xtq weights for
  the score projection G lead so the ct-outer G loop (8 PSUM accumulators,
  dt-halves with scalar/vector-alternated copies) starts ~1.5us after DMA.
- Tiles 0-3 of phase B run right after the qc=0 half of G (they only need
  the first x chunks), filling the input-DMA window with PE work; their
  ZO/EP stages are deferred. The qc=1 half of G then runs on 2 spare PSUM
  banks, and tiles 4-7 drain the deferred ZO stages two per tile.
- Within a tile the PE issue order software-pipelines:
    S(t,0) S(t,1) ZO(prev) EP(prev) TZ(t,0) S(t,2) TZ(t,1) ... ZC(t)
  so the exp latency (scalar), the z-copy, and the o-copy all hide under
  score/Z matmuls. Tile 7 processes its masked chunk third so the closing
  rowsum chain stays off the critical path, and its EP splits across the
  scalar+vector engines with per-half DMAs.
"""

import sys

if "/opt/trn_rl_repo" not in sys.path:
    sys.path.insert(0, "/opt/trn_rl_repo")

import numpy as np
import ml_dtypes

import concourse.bass as bass  # noqa: F401
import concourse.tile as tile
from concourse import bacc, mybir
from concourse.bass_utils import run_bass_kernel_spmd
from concourse.masks import make_identity

BF16 = mybir.dt.bfloat16
F32 = mybir.dt.float32
FP8 = mybir.dt.float8e4
AF = mybir.ActivationFunctionType
DR = mybir.MatmulPerfMode.DoubleRow

B, L, D = 4, 2048, 1024
P = 128
NQT = 8          # q-tiles per core, 128 rows each
SCALE = 1.0 / 32.0   # 1/sqrt(D)
NEG = -1.0e30

_CACHED = {}


def build_nc():
    nc = bacc.Bacc(None, target_bir_lowering=False)

    xt = nc.declare_dram_parameter("xt", [D, L], FP8, isOutput=False)      # x^T fp8
    xr = nc.declare_dram_parameter("xr", [L, D], BF16, isOutput=False)     # x rows
    xtq = nc.declare_dram_parameter("xtq", [D, D], BF16, isOutput=False)   # q cols of x^T
    mm_w = nc.declare_dram_parameter("mm_w", [D, D], BF16, isOutput=False)  # Wq Wk^T
    wv = nc.declare_dram_parameter("wv", [D, D], BF16, isOutput=False)
    um = nc.declare_dram_parameter("um", [P, 8], F32, isOutput=False)      # Wk bq
    bvr = nc.declare_dram_parameter("bvr", [1, D], BF16, isOutput=False)
    mask = nc.declare_dram_parameter("mask", [P, 256], F32, isOutput=False)
    out = nc.declare_dram_parameter("out", [D, D], F32, isOutput=True)

    with tile.TileContext(nc) as tc:
        with tc.tile_pool(name="persist", bufs=1) as persist:
            xt_sb = persist.tile([P, 8, L], FP8)     # x^T: [d-part, ct, token]
            xr_sb = persist.tile([P, 16, D], BF16)   # x: [tok-part, tt, d]
            gt_sb = persist.tile([P, 8, D], FP8)     # G'^T: [d-part, dt, qcol]
            xtq_sb = persist.tile([P, 8, D], BF16)
            m_sb = persist.tile([P, 8, D], BF16)
            wv_sb = persist.tile([P, 8, D], BF16)
            um_sb = persist.tile([P, 8], F32)
            bvr_sb = persist.tile([1, D], BF16)
            mask_sb = persist.tile([P, 256], F32)
            ident = persist.tile([P, P], BF16)
            ones_sb = persist.tile([1, P], BF16)
            warm_sb = persist.tile([P, 4], F32)      # warmup matmul sink

            make_identity(nc, ident)
            nc.vector.memset(ones_sb, 1.0)

            # Input streams, ordered by first use. The sync queue carries the
            # critical-path stream in exact need-order (grouped transfers to
            # stay bandwidth- not issue-bound): m/xtq-h1 for G(qc0), then the
            # x chunks tiles 0-3 consume, then xtq-h2 for G(qc1). The gpsimd
            # queue carries the late-needed bulk (wv for ZO, xt-c1/xr-hi for
            # tiles 4-7).
            for i in range(2):
                nc.sync.dma_start(out=m_sb[:, i, :],
                                  in_=mm_w[i * P:(i + 1) * P, :])
            for i in range(4):
                nc.sync.dma_start(out=xtq_sb[:, i, :512],
                                  in_=xtq[i * P:(i + 1) * P, :512])
            for i in range(2, 6):
                nc.sync.dma_start(out=m_sb[:, i, :],
                                  in_=mm_w[i * P:(i + 1) * P, :])
            for i in range(4, 8):
                nc.sync.dma_start(out=xtq_sb[:, i, :512],
                                  in_=xtq[i * P:(i + 1) * P, :512])
            for i in range(6, 8):
                nc.sync.dma_start(out=m_sb[:, i, :],
                                  in_=mm_w[i * P:(i + 1) * P, :])
            nc.sync.dma_start(out=um_sb, in_=um[:, :])
            nc.sync.dma_start(out=bvr_sb, in_=bvr[:, :])
            nc.sync.dma_start(out=mask_sb, in_=mask[:, :])
            for i in range(8):
                nc.sync.dma_start(out=xtq_sb[:, i, 512:],
                                  in_=xtq[i * P:(i + 1) * P, 512:])
            for i in range(8):
                nc.gpsimd.dma_start(out=xt_sb[:, i, :1024],
                                    in_=xt[i * P:(i + 1) * P, :1024])
            for tt in range(8):
                nc.gpsimd.dma_start(out=xr_sb[:, tt, :],
                                    in_=xr[tt * P:(tt + 1) * P, :])
            for i in range(8):
                nc.gpsimd.dma_start(out=wv_sb[:, i, :],
                                    in_=wv[i * P:(i + 1) * P, :])
            for i in range(8):
                nc.gpsimd.dma_start(out=xt_sb[:, i, 1024:],
                                    in_=xt[i * P:(i + 1) * P, 1024:])
            for tt in range(8, 16):
                nc.gpsimd.dma_start(out=xr_sb[:, tt, :],
                                    in_=xr[tt * P:(tt + 1) * P, :])

            # ---------- Phase A/B interleaved schedule -------------------
            # G = (x_q M)^T + u, computed ct-outer over PSUM accumulators so
            # the first matmul needs only the first m/xtq DMA. The qc=0
            # column-half runs first on all 8 banks; then tiles 0-1 of the
            # attention phase run (they only need qc=0 scores + the first x
            # chunks) while the rest of the inputs stream in; then the qc=1
            # half runs on 2 banks; then the remaining tiles.
            #
            # Phase-B stages per tile t (nk = 256*(t+1), nkc = #512-chunks):
            #   S(t,c): fp8 DoubleRow score matmuls into psS
            #   E(t,c): mask (last chunk) + exp -> p_sb, rowsum accum, rinv
            #   TZ(t,kt): P^T transpose + copy + 2 Z matmuls into pz
            #   ZC(t): pz -> z_sb (bf16), normalized by 1/rowsum
            #   ZO(t): Z^T transposes + O matmuls (+ bv rank-1 fold) into po
            #   EP(t): po -> o_sb (scalar), DMA out
            # PE issue order pipelines: S of the next chunk covers exp; ZO of
            # the previous tile covers the z-copy; S(t,0..1) covers ZC(t-1).
            with tc.tile_pool(name="psA", bufs=1, space="PSUM") as psA:
                # warmup chain: keeps the PE p-state ramped across the
                # initial input-DMA wait so G streams at full clock
                for rep in range(8):
                    for dc in range(8):
                        pb = psA.tile([P, 512], F32, tag=f"g{dc}",
                                      name=f"pb{dc}")
                        nc.tensor.matmul(pb[:, :P], ones_sb, ones_sb,
                                         start=True, stop=True)
                        if rep == 7 and dc < 2:
                            nc.scalar.copy(warm_sb[:, dc * 2:dc * 2 + 2],
                                           pb[:, :2])

                for dth in range(2):
                    pg = [psA.tile([P, 512], F32, tag=f"g{dt}", name=f"pg{dt}")
                          for dt in range(dth * 4, dth * 4 + 4)]
                    for ct in range(8):
                        for di, dt in enumerate(range(dth * 4, dth * 4 + 4)):
                            nc.tensor.matmul(
                                pg[di],
                                m_sb[:, ct, dt * P:(dt + 1) * P],
                                xtq_sb[:, ct, :512],
                                start=(ct == 0),
                                stop=(ct == 7),
                            )
                    for di, dt in enumerate(range(dth * 4, dth * 4 + 4)):
                        # alternate engines so the copy chain halves in wall
                        # time (scalar activation vs DVE tensor-scalar add)
                        if di % 2 == 0:
                            nc.scalar.activation(
                                gt_sb[:, dt, :512], pg[di],
                                AF.Identity, bias=um_sb[:, dt:dt + 1])
                        else:
                            nc.vector.tensor_scalar_add(
                                gt_sb[:, dt, :512], pg[di],
                                um_sb[:, dt:dt + 1])

            with tc.tile_pool(name="bwork", bufs=2) as bwork, \
                 tc.tile_pool(name="psB", bufs=2, space="PSUM") as psS, \
                 tc.tile_pool(name="psPZ", bufs=1, space="PSUM") as psPZ:
                psT = psS   # transposes share the psB pool (tag ptp)

                state = {}   # per-tile buffers shared across stage fns

                def tile_head(t):
                    state[(t, "p")] = bwork.tile([P, 2048], BF16, tag="p",
                                                 name="p")
                    state[(t, "rsum")] = bwork.tile([P, 4], F32, tag="rsum",
                                                    name="rsum")
                    state[(t, "pz")] = [
                        psPZ.tile([P, 512], F32, tag=f"pz{dc}", name=f"pz{dc}")
                        for dc in range(2)]

                def S_stage(t, c, finalize=None):
                    nk = 256 * (t + 1)
                    nkc = (nk + 511) // 512
                    if finalize is None:
                        finalize = (c == nkc - 1)
                    w = min(512, nk - c * 512)
                    ps = psS.tile([P, 512], F32, tag="s", name="ps")
                    for i in range(4):
                        nc.tensor.matmul(
                            ps[:, :w],
                            gt_sb[:, 2 * i:2 * i + 2, t * P:(t + 1) * P],
                            xt_sb[:, 2 * i:2 * i + 2, c * 512:c * 512 + w],
                            start=(i == 0),
                            stop=(i == 3),
                            perf_mode=DR,
                        )
                    # E stage issues immediately after (scalar/vector queues)
                    if c == nkc - 1:
                        nc.vector.tensor_add(ps[:, w - 256:w],
                                             ps[:, w - 256:w], mask_sb)
                    nc.scalar.activation(
                        state[(t, "p")][:, c * 512:c * 512 + w], ps[:, :w],
                        AF.Exp, scale=SCALE,
                        accum_out=state[(t, "rsum")][:, c:c + 1])
                    if finalize:
                        # rowsum -> 1/rowsum right after the final exp so the
                        # z-copy can normalize Z (folds the softmax divide)
                        rtot = bwork.tile([P, 1], F32, tag="rtot", name="rtot")
                        rinv = bwork.tile([P, 1], F32, tag="rinv", name="rinv")
                        nc.vector.reduce_sum(rtot, state[(t, "rsum")][:, :nkc],
                                             axis=mybir.AxisListType.X)
                        nc.vector.reciprocal(rinv, rtot)
                        state[(t, "rinv")] = rinv

                def TZ_stage(t, c, stop_kt=None):
                    nk = 256 * (t + 1)
                    if stop_kt is None:
                        stop_kt = nk // P - 1
                    w = min(512, nk - c * 512)
                    p_sb = state[(t, "p")]
                    pz = state[(t, "pz")]
                    for kt in range(c * 4, c * 4 + w // P):
                        ptp = psT.tile([P, P], BF16, tag="ptp", name="ptp")
                        nc.tensor.transpose(
                            ptp, p_sb[:, kt * P:(kt + 1) * P], ident)
                        pt_sb = bwork.tile([P, P], BF16, tag="pt", name="pt_sb")
                        nc.vector.tensor_copy(pt_sb, ptp)
                        for dc in range(2):
                            nc.tensor.matmul(
                                pz[dc],
                                pt_sb,
                                xr_sb[:, kt, dc * 512:(dc + 1) * 512],
                                start=(kt == 0),
                                stop=(kt == stop_kt),
                            )

                def ZC_stage(t):
                    # pz -> z_sb normalized by 1/rowsum (scalar and vector in
                    # parallel, one 512-chunk each). Tag per t%4: up to four
                    # z tiles are alive while their ZO stages are deferred.
                    z_sb = bwork.tile([P, D], BF16, tag=f"z{t % 4}",
                                      name="z_sb")
                    state[(t, "z")] = z_sb
                    rinv = state[(t, "rinv")]
                    nc.scalar.activation(z_sb[:, :512], state[(t, "pz")][0],
                                         AF.Copy, scale=rinv)
                    nc.vector.tensor_scalar_mul(z_sb[:, 512:],
                                                state[(t, "pz")][1], rinv)

                # ---- tiles 0-3 early: scores + Z only (ZO deferred) ------
                # They only need the qc=0 half of G and the first x chunks,
                # so they fill the input-DMA window with real PE work. The
                # qc=1 half of G runs on two spare PSUM banks, its quarter
                # passes interleaved so they cover the z-copy latencies.
                def g_qc1_quarter(psA2, q4):
                    pg = [psA2.tile([P, 512], F32, tag=f"h{j}", name=f"ph{j}")
                          for j in range(2)]
                    for ct in range(8):
                        for j in range(2):
                            dt = q4 * 2 + j
                            nc.tensor.matmul(
                                pg[j],
                                m_sb[:, ct, dt * P:(dt + 1) * P],
                                xtq_sb[:, ct, 512:],
                                start=(ct == 0),
                                stop=(ct == 7),
                            )
                    for j in range(2):
                        dt = q4 * 2 + j
                        if j == 0:
                            nc.scalar.activation(
                                gt_sb[:, dt, 512:], pg[j],
                                AF.Identity, bias=um_sb[:, dt:dt + 1])
                        else:
                            nc.vector.tensor_scalar_add(
                                gt_sb[:, dt, 512:], pg[j],
                                um_sb[:, dt:dt + 1])

                tile_head(0)
                S_stage(0, 0)
                tile_head(1)
                S_stage(1, 0)
                TZ_stage(0, 0)
                ZC_stage(0)
                tile_head(2)
                S_stage(2, 0)
                S_stage(2, 1)
                TZ_stage(1, 0)
                ZC_stage(1)
                tile_head(3)
                S_stage(3, 0)
                S_stage(3, 1)
                TZ_stage(2, 0)
                TZ_stage(2, 1)
                ZC_stage(2)
                TZ_stage(3, 0)
                TZ_stage(3, 1)
                ZC_stage(3)
                with tc.tile_pool(name="psA2", bufs=1, space="PSUM") as psA2:
                    for q4 in range(4):
                        g_qc1_quarter(psA2, q4)

                # ---- remaining tiles, with ZO/EP pipelined one tile back --
                with tc.tile_pool(name="psC", bufs=1, space="PSUM") as psC:

                    def ZO_stage(t):
                        z_sb = state[(t, "z")]
                        po = [psC.tile([P, 512], F32, tag=f"po{dc}",
                                       name=f"po{dc}")
                              for dc in range(2)]
                        state[(t, "po")] = po
                        for cc in range(8):
                            ztp = psT.tile([P, P], BF16, tag="ptp", name="ztp")
                            nc.tensor.transpose(
                                ztp, z_sb[:, cc * P:(cc + 1) * P], ident)
                            zt_sb = bwork.tile([P, P], BF16, tag="zt",
                                               name="zt_sb")
                            nc.vector.tensor_copy(zt_sb, ztp)
                            for dc in range(2):
                                nc.tensor.matmul(
                                    po[dc],
                                    zt_sb,
                                    wv_sb[:, cc, dc * 512:(dc + 1) * 512],
                                    start=(cc == 0),
                                    stop=(cc == 7),
                                )
                            if cc == 0:
                                # fold the output bias: po += 1 (x) bv (K=1)
                                for dc in range(2):
                                    nc.tensor.matmul(
                                        po[dc], ones_sb,
                                        bvr_sb[:, dc * 512:(dc + 1) * 512],
                                        start=False, stop=False)

                    def EP_stage(t, final=False):
                        # po is the finished output (normalized, biased);
                        # stage through SBUF on scalar (keeps the vector
                        # queue free for the latency-critical pt copies) and
                        # DMA out. The last tile has an idle vector queue, so
                        # split across engines with per-half DMAs instead.
                        po = state[(t, "po")]
                        o_sb = bwork.tile([P, D], F32, tag="o", name="o_sb")
                        if final:
                            nc.scalar.copy(o_sb[:, :512], po[0])
                            nc.sync.dma_start(
                                out=out[t * P:(t + 1) * P, :512],
                                in_=o_sb[:, :512])
                            nc.vector.tensor_copy(o_sb[:, 512:], po[1])
                            nc.sync.dma_start(
                                out=out[t * P:(t + 1) * P, 512:],
                                in_=o_sb[:, 512:])
                        else:
                            for dc in range(2):
                                nc.scalar.copy(
                                    o_sb[:, dc * 512:(dc + 1) * 512], po[dc])
                            nc.sync.dma_start(out=out[t * P:(t + 1) * P, :],
                                              in_=o_sb)

                    # Deferred ZO/EP stages drain two-per-tile while the
                    # remaining tiles' score/Z work keeps the PE fed.
                    pending = [0, 1, 2, 3]
                    ZO_stage(pending.pop(0))
                    EP_stage(0)
                    for t in range(4, NQT - 1):
                        nk = 256 * (t + 1)
                        nkc = (nk + 511) // 512
                        tile_head(t)
                        S_stage(t, 0)
                        S_stage(t, 1)
                        done_s = 2
                        zo = pending.pop(0)
                        ZO_stage(zo)
                        EP_stage(zo)
                        for c in range(nkc):
                            TZ_stage(t, c)
                            if done_s < nkc:
                                S_stage(t, done_s)
                                done_s += 1
                            if c == 1 and t <= 5 and pending:
                                zo = pending.pop(0)
                                ZO_stage(zo)
                                EP_stage(zo)
                        ZC_stage(t)
                        pending.append(t)
                    # tile 7 processes its masked chunk (3) third and the
                    # plain chunk 2 last, so the end-of-tile rowsum chain
                    # (mask -> exp -> reduce -> recip -> z-copy) is off the
                    # critical path; the Z accumulation stops at kt=11.
                    t = NQT - 1
                    tile_head(t)
                    S_stage(t, 0)
                    S_stage(t, 1)
                    zo = pending.pop(0)
                    ZO_stage(zo)
                    EP_stage(zo)
                    TZ_stage(t, 0, stop_kt=11)
                    S_stage(t, 3, finalize=False)
                    TZ_stage(t, 1, stop_kt=11)
                    S_stage(t, 2, finalize=True)
                    TZ_stage(t, 3, stop_kt=11)
                    TZ_stage(t, 2, stop_kt=11)
                    ZC_stage(t)
                    pending.append(t)
                    while pending:
                        zo = pending.pop(0)
                        ZO_stage(zo)
                        EP_stage(zo, final=not pending)

    nc.finalize()
    return nc


def _prep_inputs(x, wq, bq, wk, bk, wv, bv):
    bf = ml_dtypes.bfloat16
    f8 = ml_dtypes.float8_e4m3
    wq32 = np.asarray(wq, np.float32)
    wk32 = np.asarray(wk, np.float32)
    m_host = (wq32 @ wk32.T).astype(bf)                 # Wq Wk^T
    u_host = (wk32 @ np.asarray(bq, np.float32))        # Wk bq, [D]
    um = np.ascontiguousarray(u_host.reshape(8, P).T).astype(np.float32)
    wv_b = np.ascontiguousarray(wv, np.float32).astype(bf)
    bvr = np.asarray(bv, np.float32).reshape(1, D).astype(bf)

    i = np.arange(P)[:, None]
    j = np.arange(256)[None, :]
    masks = [np.where(j <= i + P * h, 0.0, NEG).astype(np.float32)
             for h in range(2)]

    in_maps = []
    for core in range(8):
        b, h = core // 2, core % 2
        xb = np.asarray(x[b], np.float32)
        xT = np.ascontiguousarray(xb.T)
        xR = xb.astype(bf)
        qcols = (np.arange(8)[:, None] * 2 + h) * P + np.arange(P)[None, :]
        xTq = np.ascontiguousarray(xT[:, qcols.ravel()]).astype(bf)
        in_maps.append({
            "xt": xT.astype(f8), "xr": xR, "xtq": xTq, "mm_w": m_host,
            "wv": wv_b, "um": um, "bvr": bvr, "mask": masks[h],
        })
    return in_maps


def kernel(x, wq, bq, wk, bk, wv, bv, _trace=False, _trace_kwargs=None):
    if "nc" not in _CACHED:
        _CACHED["nc"] = build_nc()
    nc = _CACHED["nc"]
    in_maps = _prep_inputs(x, wq, bq, wk, bk, wv, bv)
    kw = {}
    if _trace:
        kw = dict(trace=True, **(_trace_kwargs or {}))
    res = run_bass_kernel_spmd(nc, in_maps, list(range(8)), **kw)
    out = np.empty((B, L, D), np.float32)
    for core in range(8):
        b, h = core // 2, core % 2
        o = np.asarray(res.results[core]["out"], np.float32)
        out[b].reshape(16, P, D)[h::2] = o.reshape(NQT, P, D)
    if _trace:
        _CACHED["last_results"] = res
    return out


# revision 28
# speedup vs baseline: 1.0016x; 1.0004x over previous
"""MiniCausalAttention on 8 NeuronCores (Trainium2, Bass/Tile).

Problem: x[4,2048,1024] fp32; q/k/v = x@w+b; causal softmax(q k^T/sqrt(D)) @ v.

Sharding: 8 cores = (batch b in 0..3) x (half h in 0..1). Core (b,h) handles
query tiles t' = 2t+h for t in 0..7 (interleaved 128-row tiles), so every
core sees the SAME set of causal key-extents nk(t) = 256*(t+1) -> one SPMD
program, perfectly balanced.

Projection reassociation (exact algebra, host-precomputed M = Wq Wk^T and
u = Wk bq):
  scores  S = q k^T = x_q M x^T + 1 (x) (x u)^T  (+ per-query terms that
          cancel in softmax and are dropped)
  output  O = P_norm v = [(P x) Wv] / rowsum + bv
so neither K nor V is ever materialized. The key-bias u·x_k term is folded
into G^T = (x_q M)^T at PSUM->SBUF copy time as a per-partition activation
bias (G'^T[d,q] = G^T[d,q] + u[d]); 1/rowsum is folded into the Z copy; the
output bias bv is folded into the O accumulation as a rank-1 K=1 matmul.

Precision: G is computed in bf16 (scores accuracy), then stored fp8-e4m3;
x^T is shipped fp8 -> the S matmul runs fp8 DoubleRow (K=256/instruction,
1 col/cycle = 2x bf16 FLOP rate) costing ~1.3e-2 rel err total (tol 2e-2,
validated in numpy sim; fp8 anywhere in the P*V path fails tolerance, and
fp8 multi-plane residual tricks lose to bf16 since DR is 1 cyc/col).
P/Z/O stay bf16; PSUM accumulation fp32; softmax statistics fp32.

Scheduling (PE is the bottleneck; it streams 1 col/cycle at 2.0-2.4GHz with
LDWEIGHTS fully hidden, so only column count and stalls matter):
- Inputs stream in first-use order on two DMA queues; the m/xtq weights for
  the score projection G lead so the ct-outer G loop (8 PSUM accumulators,
  dt-halves with scalar/vector-alternated copies) starts ~1.5us after DMA.
- Tiles 0-3 of phase B run right after the qc=0 half of G (they only need
  the first x chunks), filling the input-DMA window with PE work; their
  ZO/EP stages are deferred. The qc=1 half of G then runs on 2 spare PSUM
  banks, and tiles 4-7 drain the deferred ZO stages two per tile.
- Within a tile the PE issue order software-pipelines:
    S(t,0) S(t,1) ZO(prev) EP(prev) TZ(t,0) S(t,2) TZ(t,1) ... ZC(t)
  so the exp latency (scalar), the z-copy, and the o-copy all hide under
  score/Z matmuls. Tile 7 processes its masked chunk third so the closing
  rowsum chain stays off the critical path, and its EP splits across the
  scalar+vector engines with per-half DMAs.
"""

import sys

if "/opt/trn_rl_repo" not in sys.path:
    sys.path.insert(0, "/opt/trn_rl_repo")

import numpy as np
import ml_dtypes

import concourse.bass as bass  # noqa: F401
import concourse.tile as tile
from concourse import bacc, mybir
from concourse.bass_utils import run_bass_kernel_spmd
from concourse.masks import make_identity

BF16 = mybir.dt.bfloat16
F32 = mybir.dt.float32
FP8 = mybir.dt.float8e4
AF = mybir.ActivationFunctionType
DR = mybir.MatmulPerfMode.DoubleRow

B, L, D = 4, 2048, 1024
P = 128
NQT = 8          # q-tiles per core, 128 rows each
SCALE = 1.0 / 32.0   # 1/sqrt(D)
NEG = -1.0e30

_CACHED = {}


def build_nc():
    nc = bacc.Bacc(None, target_bir_lowering=False)

    xt = nc.declare_dram_parameter("xt", [D, L], FP8, isOutput=False)      # x^T fp8
    xr = nc.declare_dram_parameter("xr", [L, D], BF16, isOutput=False)     # x rows
    xtq = nc.declare_dram_parameter("xtq", [D, D], BF16, isOutput=False)   # q cols of x^T
    mm_w = nc.declare_dram_parameter("mm_w", [D, D], BF16, isOutput=False)  # Wq Wk^T
    wv = nc.declare_dram_parameter("wv", [D, D], BF16, isOutput=False)
    um = nc.declare_dram_parameter("um", [P, 8], F32, isOutput=False)      # Wk bq
    bvr = nc.declare_dram_parameter("bvr", [1, D], BF16, isOutput=False)
    mask = nc.declare_dram_parameter("mask", [P, 256], F32, isOutput=False)
    out = nc.declare_dram_parameter("out", [D, D], F32, isOutput=True)

    with tile.TileContext(nc) as tc:
        with tc.tile_pool(name="persist", bufs=1) as persist:
            xt_sb = persist.tile([P, 8, L], FP8)     # x^T: [d-part, ct, token]
            xr_sb = persist.tile([P, 16, D], BF16)   # x: [tok-part, tt, d]
            gt_sb = persist.tile([P, 8, D], FP8)     # G'^T: [d-part, dt, qcol]
            xtq_sb = persist.tile([P, 8, D], BF16)
            m_sb = persist.tile([P, 8, D], BF16)
            wv_sb = persist.tile([P, 8, D], BF16)
            um_sb = persist.tile([P, 8], F32)
            bvr_sb = persist.tile([1, D], BF16)
            mask_sb = persist.tile([P, 256], F32)
            ident = persist.tile([P, P], BF16)
            ones_sb = persist.tile([1, P], BF16)
            warm_sb = persist.tile([P, 4], F32)      # warmup matmul sink

            make_identity(nc, ident)
            nc.vector.memset(ones_sb, 1.0)

            # Input streams, ordered by first use. The sync queue carries the
            # critical-path stream in exact need-order (grouped transfers to
            # stay bandwidth- not issue-bound): m/xtq-h1 for G(qc0), then the
            # x chunks tiles 0-3 consume, then xtq-h2 for G(qc1). The gpsimd
            # queue carries the late-needed bulk (wv for ZO, xt-c1/xr-hi for
            # tiles 4-7).
            for i in range(2):
                nc.sync.dma_start(out=m_sb[:, i, :],
                                  in_=mm_w[i * P:(i + 1) * P, :])
            for i in range(4):
                nc.sync.dma_start(out=xtq_sb[:, i, :512],
                                  in_=xtq[i * P:(i + 1) * P, :512])
            for i in range(2, 6):
                nc.sync.dma_start(out=m_sb[:, i, :],
                                  in_=mm_w[i * P:(i + 1) * P, :])
            for i in range(4, 8):
                nc.sync.dma_start(out=xtq_sb[:, i, :512],
                                  in_=xtq[i * P:(i + 1) * P, :512])
            for i in range(6, 8):
                nc.sync.dma_start(out=m_sb[:, i, :],
                                  in_=mm_w[i * P:(i + 1) * P, :])
            nc.sync.dma_start(out=um_sb, in_=um[:, :])
            nc.sync.dma_start(out=bvr_sb, in_=bvr[:, :])
            nc.sync.dma_start(out=mask_sb, in_=mask[:, :])
            for i in range(8):
                nc.sync.dma_start(out=xtq_sb[:, i, 512:],
                                  in_=xtq[i * P:(i + 1) * P, 512:])
            for i in range(8):
                nc.gpsimd.dma_start(out=xt_sb[:, i, :1024],
                                    in_=xt[i * P:(i + 1) * P, :1024])
            for tt in range(8):
                nc.gpsimd.dma_start(out=xr_sb[:, tt, :],
                                    in_=xr[tt * P:(tt + 1) * P, :])
            for i in range(8):
                nc.gpsimd.dma_start(out=wv_sb[:, i, :],
                                    in_=wv[i * P:(i + 1) * P, :])
            for i in range(8):
                nc.gpsimd.dma_start(out=xt_sb[:, i, 1024:],
                                    in_=xt[i * P:(i + 1) * P, 1024:])
            for tt in range(8, 16):
                nc.gpsimd.dma_start(out=xr_sb[:, tt, :],
                                    in_=xr[tt * P:(tt + 1) * P, :])

            # ---------- Phase A/B interleaved schedule -------------------
            # G = (x_q M)^T + u, computed ct-outer over PSUM accumulators so
            # the first matmul needs only the first m/xtq DMA. The qc=0
            # column-half runs first on all 8 banks; then tiles 0-1 of the
            # attention phase run (they only need qc=0 scores + the first x
            # chunks) while the rest of the inputs stream in; then the qc=1
            # half runs on 2 banks; then the remaining tiles.
            #
            # Phase-B stages per tile t (nk = 256*(t+1), nkc = #512-chunks):
            #   S(t,c): fp8 DoubleRow score matmuls into psS
            #   E(t,c): mask (last chunk) + exp -> p_sb, rowsum accum, rinv
            #   TZ(t,kt): P^T transpose + copy + 2 Z matmuls into pz
            #   ZC(t): pz -> z_sb (bf16), normalized by 1/rowsum
            #   ZO(t): Z^T transposes + O matmuls (+ bv rank-1 fold) into po
            #   EP(t): po -> o_sb (scalar), DMA out
            # PE issue order pipelines: S of the next chunk covers exp; ZO of
            # the previous tile covers the z-copy; S(t,0..1) covers ZC(t-1).
            with tc.tile_pool(name="psA", bufs=1, space="PSUM") as psA:
                # warmup chain: keeps the PE p-state ramped across the
                # initial input-DMA wait so G streams at full clock
                for rep in range(6):
                    for dc in range(8):
                        pb = psA.tile([P, 512], F32, tag=f"g{dc}",
                                      name=f"pb{dc}")
                        nc.tensor.matmul(pb[:, :P], ones_sb, ones_sb,
                                         start=True, stop=True)
                        if rep == 5 and dc < 2:
                            nc.scalar.copy(warm_sb[:, dc * 2:dc * 2 + 2],
                                           pb[:, :2])

                for dth in range(2):
                    pg = [psA.tile([P, 512], F32, tag=f"g{dt}", name=f"pg{dt}")
                          for dt in range(dth * 4, dth * 4 + 4)]
                    for ct in range(8):
                        for di, dt in enumerate(range(dth * 4, dth * 4 + 4)):
                            nc.tensor.matmul(
                                pg[di],
                                m_sb[:, ct, dt * P:(dt + 1) * P],
                                xtq_sb[:, ct, :512],
                                start=(ct == 0),
                                stop=(ct == 7),
                            )
                    for di, dt in enumerate(range(dth * 4, dth * 4 + 4)):
                        # alternate engines so the copy chain halves in wall
                        # time (scalar activation vs DVE tensor-scalar add)
                        if di % 2 == 0:
                            nc.scalar.activation(
                                gt_sb[:, dt, :512], pg[di],
                                AF.Identity, bias=um_sb[:, dt:dt + 1])
                        else:
                            nc.vector.tensor_scalar_add(
                                gt_sb[:, dt, :512], pg[di],
                                um_sb[:, dt:dt + 1])

            with tc.tile_pool(name="bwork", bufs=2) as bwork, \
                 tc.tile_pool(name="psB", bufs=2, space="PSUM") as psS, \
                 tc.tile_pool(name="psPZ", bufs=1, space="PSUM") as psPZ:
                psT = psS   # transposes share the psB pool (tag ptp)

                state = {}   # per-tile buffers shared across stage fns

                def tile_head(t):
                    state[(t, "p")] = bwork.tile([P, 2048], BF16, tag="p",
                                                 name="p")
                    state[(t, "rsum")] = bwork.tile([P, 4], F32, tag="rsum",
                                                    name="rsum")
                    state[(t, "pz")] = [
                        psPZ.tile([P, 512], F32, tag=f"pz{dc}", name=f"pz{dc}")
                        for dc in range(2)]

                def S_stage(t, c, finalize=None):
                    nk = 256 * (t + 1)
                    nkc = (nk + 511) // 512
                    if finalize is None:
                        finalize = (c == nkc - 1)
                    w = min(512, nk - c * 512)
                    ps = psS.tile([P, 512], F32, tag="s", name="ps")
                    for i in range(4):
                        nc.tensor.matmul(
                            ps[:, :w],
                            gt_sb[:, 2 * i:2 * i + 2, t * P:(t + 1) * P],
                            xt_sb[:, 2 * i:2 * i + 2, c * 512:c * 512 + w],
                            start=(i == 0),
                            stop=(i == 3),
                            perf_mode=DR,
                        )
                    # E stage issues immediately after (scalar/vector queues)
                    if c == nkc - 1:
                        nc.vector.tensor_add(ps[:, w - 256:w],
                                             ps[:, w - 256:w], mask_sb)
                    nc.scalar.activation(
                        state[(t, "p")][:, c * 512:c * 512 + w], ps[:, :w],
                        AF.Exp, scale=SCALE,
                        accum_out=state[(t, "rsum")][:, c:c + 1])
                    if finalize:
                        # rowsum -> 1/rowsum right after the final exp so the
                        # z-copy can normalize Z (folds the softmax divide)
                        rtot = bwork.tile([P, 1], F32, tag="rtot", name="rtot")
                        rinv = bwork.tile([P, 1], F32, tag="rinv", name="rinv")
                        nc.vector.reduce_sum(rtot, state[(t, "rsum")][:, :nkc],
                                             axis=mybir.AxisListType.X)
                        nc.vector.reciprocal(rinv, rtot)
                        state[(t, "rinv")] = rinv

                def TZ_stage(t, c, stop_kt=None):
                    nk = 256 * (t + 1)
                    if stop_kt is None:
                        stop_kt = nk // P - 1
                    w = min(512, nk - c * 512)
                    p_sb = state[(t, "p")]
                    pz = state[(t, "pz")]
                    for kt in range(c * 4, c * 4 + w // P):
                        ptp = psT.tile([P, P], BF16, tag="ptp", name="ptp")
                        nc.tensor.transpose(
                            ptp, p_sb[:, kt * P:(kt + 1) * P], ident)
                        pt_sb = bwork.tile([P, P], BF16, tag="pt", name="pt_sb")
                        nc.vector.tensor_copy(pt_sb, ptp)
                        for dc in range(2):
                            nc.tensor.matmul(
                                pz[dc],
                                pt_sb,
                                xr_sb[:, kt, dc * 512:(dc + 1) * 512],
                                start=(kt == 0),
                                stop=(kt == stop_kt),
                            )

                def ZC_stage(t):
                    # pz -> z_sb normalized by 1/rowsum (scalar and vector in
                    # parallel, one 512-chunk each). Tag per t%4: up to four
                    # z tiles are alive while their ZO stages are deferred.
                    z_sb = bwork.tile([P, D], BF16, tag=f"z{t % 4}",
                                      name="z_sb")
                    state[(t, "z")] = z_sb
                    rinv = state[(t, "rinv")]
                    nc.scalar.activation(z_sb[:, :512], state[(t, "pz")][0],
                                         AF.Copy, scale=rinv)
                    nc.vector.tensor_scalar_mul(z_sb[:, 512:],
                                                state[(t, "pz")][1], rinv)

                # ---- tiles 0-3 early: scores + Z only (ZO deferred) ------
                # They only need the qc=0 half of G and the first x chunks,
                # so they fill the input-DMA window with real PE work. The
                # qc=1 half of G runs on two spare PSUM banks, its quarter
                # passes interleaved so they cover the z-copy latencies.
                def g_qc1_quarter(psA2, q4):
                    pg = [psA2.tile([P, 512], F32, tag=f"h{j}", name=f"ph{j}")
                          for j in range(2)]
                    for ct in range(8):
                        for j in range(2):
                            dt = q4 * 2 + j
                            nc.tensor.matmul(
                                pg[j],
                                m_sb[:, ct, dt * P:(dt + 1) * P],
                                xtq_sb[:, ct, 512:],
                                start=(ct == 0),
                                stop=(ct == 7),
                            )
                    for j in range(2):
                        dt = q4 * 2 + j
                        if j == 0:
                            nc.scalar.activation(
                                gt_sb[:, dt, 512:], pg[j],
                                AF.Identity, bias=um_sb[:, dt:dt + 1])
                        else:
                            nc.vector.tensor_scalar_add(
                                gt_sb[:, dt, 512:], pg[j],
                                um_sb[:, dt:dt + 1])

                tile_head(0)
                S_stage(0, 0)
                tile_head(1)
                S_stage(1, 0)
                TZ_stage(0, 0)
                ZC_stage(0)
                tile_head(2)
                S_stage(2, 0)
                S_stage(2, 1)
                TZ_stage(1, 0)
                ZC_stage(1)
                tile_head(3)
                S_stage(3, 0)
                S_stage(3, 1)
                TZ_stage(2, 0)
                TZ_stage(2, 1)
                ZC_stage(2)
                TZ_stage(3, 0)
                TZ_stage(3, 1)
                ZC_stage(3)
                with tc.tile_pool(name="psA2", bufs=1, space="PSUM") as psA2:
                    for q4 in range(4):
                        g_qc1_quarter(psA2, q4)

                # ---- remaining tiles, with ZO/EP pipelined one tile back --
                with tc.tile_pool(name="psC", bufs=1, space="PSUM") as psC:

                    def ZO_stage(t):
                        z_sb = state[(t, "z")]
                        po = [psC.tile([P, 512], F32, tag=f"po{dc}",
                                       name=f"po{dc}")
                              for dc in range(2)]
                        state[(t, "po")] = po
                        for cc in range(8):
                            ztp = psT.tile([P, P], BF16, tag="ptp", name="ztp")
                            nc.tensor.transpose(
                                ztp, z_sb[:, cc * P:(cc + 1) * P], ident)
                            zt_sb = bwork.tile([P, P], BF16, tag="zt",
                                               name="zt_sb")
                            nc.vector.tensor_copy(zt_sb, ztp)
                            for dc in range(2):
                                nc.tensor.matmul(
                                    po[dc],
                                    zt_sb,
                                    wv_sb[:, cc, dc * 512:(dc + 1) * 512],
                                    start=(cc == 0),
                                    stop=(cc == 7),
                                )
                            if cc == 0:
                                # fold the output bias: po += 1 (x) bv (K=1)
                                for dc in range(2):
                                    nc.tensor.matmul(
                                        po[dc], ones_sb,
                                        bvr_sb[:, dc * 512:(dc + 1) * 512],
                                        start=False, stop=False)

                    def EP_stage(t, final=False):
                        # po is the finished output (normalized, biased);
                        # stage through SBUF on scalar (keeps the vector
                        # queue free for the latency-critical pt copies) and
                        # DMA out. The last tile has an idle vector queue, so
                        # split across engines with per-half DMAs instead.
                        po = state[(t, "po")]
                        o_sb = bwork.tile([P, D], F32, tag="o", name="o_sb")
                        if final:
                            nc.scalar.copy(o_sb[:, :512], po[0])
                            nc.sync.dma_start(
                                out=out[t * P:(t + 1) * P, :512],
                                in_=o_sb[:, :512])
                            nc.vector.tensor_copy(o_sb[:, 512:], po[1])
                            nc.sync.dma_start(
                                out=out[t * P:(t + 1) * P, 512:],
                                in_=o_sb[:, 512:])
                        else:
                            for dc in range(2):
                                nc.scalar.copy(
                                    o_sb[:, dc * 512:(dc + 1) * 512], po[dc])
                            nc.sync.dma_start(out=out[t * P:(t + 1) * P, :],
                                              in_=o_sb)

                    # Deferred ZO/EP stages drain two-per-tile while the
                    # remaining tiles' score/Z work keeps the PE fed.
                    pending = [0, 1, 2, 3]
                    ZO_stage(pending.pop(0))
                    EP_stage(0)
                    for t in range(4, NQT - 1):
                        nk = 256 * (t + 1)
                        nkc = (nk + 511) // 512
                        tile_head(t)
                        S_stage(t, 0)
                        S_stage(t, 1)
                        done_s = 2
                        zo = pending.pop(0)
                        ZO_stage(zo)
                        EP_stage(zo)
                        for c in range(nkc):
                            TZ_stage(t, c)
                            if done_s < nkc:
                                S_stage(t, done_s)
                                done_s += 1
                            if c == 1 and t <= 5 and pending:
                                zo = pending.pop(0)
                                ZO_stage(zo)
                                EP_stage(zo)
                        ZC_stage(t)
                        pending.append(t)
                    # tile 7 processes its masked chunk (3) third and the
                    # plain chunk 2 last, so the end-of-tile rowsum chain
                    # (mask -> exp -> reduce -> recip -> z-copy) is off the
                    # critical path; the Z accumulation stops at kt=11.
                    t = NQT - 1
                    tile_head(t)
                    S_stage(t, 0)
                    S_stage(t, 1)
                    zo = pending.pop(0)
                    ZO_stage(zo)
                    EP_stage(zo)
                    TZ_stage(t, 0, stop_kt=11)
                    S_stage(t, 3, finalize=False)
                    TZ_stage(t, 1, stop_kt=11)
                    S_stage(t, 2, finalize=True)
                    TZ_stage(t, 3, stop_kt=11)
                    TZ_stage(t, 2, stop_kt=11)
                    ZC_stage(t)
                    pending.append(t)
                    while pending:
                        zo = pending.pop(0)
                        ZO_stage(zo)
                        EP_stage(zo, final=not pending)

    nc.finalize()
    return nc


def _prep_inputs(x, wq, bq, wk, bk, wv, bv):
    bf = ml_dtypes.bfloat16
    f8 = ml_dtypes.float8_e4m3
    wq32 = np.asarray(wq, np.float32)
    wk32 = np.asarray(wk, np.float32)
    m_host = (wq32 @ wk32.T).astype(bf)                 # Wq Wk^T
    u_host = (wk32 @ np.asarray(bq, np.float32))        # Wk bq, [D]
    um = np.ascontiguousarray(u_host.reshape(8, P).T).astype(np.float32)
    wv_b = np.ascontiguousarray(wv, np.float32).astype(bf)
    bvr = np.asarray(bv, np.float32).reshape(1, D).astype(bf)

    i = np.arange(P)[:, None]
    j = np.arange(256)[None, :]
    masks = [np.where(j <= i + P * h, 0.0, NEG).astype(np.float32)
             for h in range(2)]

    in_maps = []
    for core in range(8):
        b, h = core // 2, core % 2
        xb = np.asarray(x[b], np.float32)
        xT = np.ascontiguousarray(xb.T)
        xR = xb.astype(bf)
        qcols = (np.arange(8)[:, None] * 2 + h) * P + np.arange(P)[None, :]
        xTq = np.ascontiguousarray(xT[:, qcols.ravel()]).astype(bf)
        in_maps.append({
            "xt": xT.astype(f8), "xr": xR, "xtq": xTq, "mm_w": m_host,
            "wv": wv_b, "um": um, "bvr": bvr, "mask": masks[h],
        })
    return in_maps


def kernel(x, wq, bq, wk, bk, wv, bv, _trace=False, _trace_kwargs=None):
    if "nc" not in _CACHED:
        _CACHED["nc"] = build_nc()
    nc = _CACHED["nc"]
    in_maps = _prep_inputs(x, wq, bq, wk, bk, wv, bv)
    kw = {}
    if _trace:
        kw = dict(trace=True, **(_trace_kwargs or {}))
    res = run_bass_kernel_spmd(nc, in_maps, list(range(8)), **kw)
    out = np.empty((B, L, D), np.float32)
    for core in range(8):
        b, h = core // 2, core % 2
        o = np.asarray(res.results[core]["out"], np.float32)
        out[b].reshape(16, P, D)[h::2] = o.reshape(NQT, P, D)
    if _trace:
        _CACHED["last_results"] = res
    return out


# revision 29
# speedup vs baseline: 1.0255x; 1.0239x over previous
"""MiniCausalAttention on 8 NeuronCores (Trainium2, Bass/Tile).

Problem: x[4,2048,1024] fp32; q/k/v = x@w+b; causal softmax(q k^T/sqrt(D)) @ v.

Sharding: 8 cores = (batch b in 0..3) x (half h in 0..1). Core (b,h) handles
query tiles t' = 2t+h for t in 0..7 (interleaved 128-row tiles), so every
core sees the SAME set of causal key-extents nk(t) = 256*(t+1) -> one SPMD
program, perfectly balanced.

Projection reassociation (exact algebra, host-precomputed M = Wq Wk^T and
u = Wk bq):
  scores  S = q k^T = x_q M x^T + 1 (x) (x u)^T  (+ per-query terms that
          cancel in softmax and are dropped)
  output  O = P_norm v = [(P x) Wv] / rowsum + bv
so neither K nor V is ever materialized. The key-bias u·x_k term is folded
into G^T = (x_q M)^T at PSUM->SBUF copy time as a per-partition activation
bias (G'^T[d,q] = G^T[d,q] + u[d]); 1/rowsum is folded into the Z copy; the
output bias bv is folded into the O accumulation as a rank-1 K=1 matmul.

Precision: G is computed in bf16 (scores accuracy), then stored fp8-e4m3;
x^T is shipped fp8 -> the S matmul runs fp8 DoubleRow (K=256/instruction,
1 col/cycle = 2x bf16 FLOP rate) costing ~1.3e-2 rel err total (tol 2e-2,
validated in numpy sim; fp8 anywhere in the P*V path fails tolerance, and
fp8 multi-plane residual tricks lose to bf16 since DR is 1 cyc/col).
P/Z/O stay bf16; PSUM accumulation fp32; softmax statistics fp32.

Scheduling (PE is the bottleneck; it streams 1 col/cycle at 2.0-2.4GHz with
LDWEIGHTS fully hidden, so only column count and stalls matter):
- Inputs stream in first-use order on two DMA queues; the m/xtq weights for
  the score projection G lead so the ct-outer G loop (8 PSUM accumulators,
  dt-halves with scalar/vector-alternated copies) starts ~1.5us after DMA.
- Tiles 0-3 of phase B run right after the qc=0 half of G (they only need
  the first x chunks), filling the input-DMA window with PE work; their
  ZO/EP stages are deferred. The qc=1 half of G then runs on 2 spare PSUM
  banks, and tiles 4-7 drain the deferred ZO stages two per tile.
- Within a tile the PE issue order software-pipelines:
    S(t,0) S(t,1) ZO(prev) EP(prev) TZ(t,0) S(t,2) TZ(t,1) ... ZC(t)
  so the exp latency (scalar), the z-copy, and the o-copy all hide under
  score/Z matmuls. Tile 7 processes its masked chunk third so the closing
  rowsum chain stays off the critical path, and its EP splits across the
  scalar+vector engines with per-half DMAs.
"""

import sys

if "/opt/trn_rl_repo" not in sys.path:
    sys.path.insert(0, "/opt/trn_rl_repo")

import numpy as np
import ml_dtypes

import concourse.bass as bass  # noqa: F401
import concourse.tile as tile
from concourse import bacc, mybir
from concourse.bass_utils import run_bass_kernel_spmd
from concourse.masks import make_identity

BF16 = mybir.dt.bfloat16
F32 = mybir.dt.float32
FP8 = mybir.dt.float8e4
AF = mybir.ActivationFunctionType
DR = mybir.MatmulPerfMode.DoubleRow

B, L, D = 4, 2048, 1024
P = 128
NQT = 8          # q-tiles per core, 128 rows each
SCALE = 1.0 / 32.0   # 1/sqrt(D)
NEG = -1.0e30

_CACHED = {}


def build_nc():
    nc = bacc.Bacc(None, target_bir_lowering=False)

    xt = nc.declare_dram_parameter("xt", [D, L], FP8, isOutput=False)      # x^T fp8
    xr = nc.declare_dram_parameter("xr", [L, D], BF16, isOutput=False)     # x rows
    xtq = nc.declare_dram_parameter("xtq", [D, D], BF16, isOutput=False)   # q cols of x^T
    mm_w = nc.declare_dram_parameter("mm_w", [D, D], BF16, isOutput=False)  # Wq Wk^T
    wv = nc.declare_dram_parameter("wv", [D, D], BF16, isOutput=False)
    um = nc.declare_dram_parameter("um", [P, 8], F32, isOutput=False)      # Wk bq
    bvr = nc.declare_dram_parameter("bvr", [1, D], BF16, isOutput=False)
    mask = nc.declare_dram_parameter("mask", [P, 256], F32, isOutput=False)
    out = nc.declare_dram_parameter("out", [D, D], F32, isOutput=True)

    with tile.TileContext(nc) as tc:
        with tc.tile_pool(name="persist", bufs=1) as persist:
            xt_sb = persist.tile([P, 8, L], FP8)     # x^T: [d-part, ct, token]
            xr_sb = persist.tile([P, 16, D], BF16)   # x: [tok-part, tt, d]
            gt_sb = persist.tile([P, 8, D], FP8)     # G'^T: [d-part, dt, qcol]
            xtq_sb = persist.tile([P, 8, D], BF16)
            m_sb = persist.tile([P, 8, D], BF16)
            wv_sb = persist.tile([P, 8, D], BF16)
            um_sb = persist.tile([P, 8], F32)
            bvr_sb = persist.tile([1, D], BF16)
            mask_sb = persist.tile([P, 256], F32)
            ident = persist.tile([P, P], BF16)
            ones_sb = persist.tile([1, P], BF16)
            warm_sb = persist.tile([P, 4], F32)      # warmup matmul sink
            bvb_sb = persist.tile([P, D], F32)       # broadcast bias 1 (x) bv

            make_identity(nc, ident)
            nc.vector.memset(ones_sb, 1.0)

            # Input streams, ordered by first use. The sync queue carries the
            # critical-path stream in exact need-order (grouped transfers to
            # stay bandwidth- not issue-bound): m/xtq-h1 for G(qc0), then the
            # x chunks tiles 0-3 consume, then xtq-h2 for G(qc1). The gpsimd
            # queue carries the late-needed bulk (wv for ZO, xt-c1/xr-hi for
            # tiles 4-7).
            for i in range(2):
                nc.sync.dma_start(out=m_sb[:, i, :],
                                  in_=mm_w[i * P:(i + 1) * P, :])
            for i in range(4):
                nc.sync.dma_start(out=xtq_sb[:, i, :512],
                                  in_=xtq[i * P:(i + 1) * P, :512])
            for i in range(2, 6):
                nc.sync.dma_start(out=m_sb[:, i, :],
                                  in_=mm_w[i * P:(i + 1) * P, :])
            for i in range(4, 8):
                nc.sync.dma_start(out=xtq_sb[:, i, :512],
                                  in_=xtq[i * P:(i + 1) * P, :512])
            for i in range(6, 8):
                nc.sync.dma_start(out=m_sb[:, i, :],
                                  in_=mm_w[i * P:(i + 1) * P, :])
            nc.sync.dma_start(out=um_sb, in_=um[:, :])
            nc.sync.dma_start(out=bvr_sb, in_=bvr[:, :])
            nc.sync.dma_start(out=mask_sb, in_=mask[:, :])
            for i in range(8):
                nc.sync.dma_start(out=xtq_sb[:, i, 512:],
                                  in_=xtq[i * P:(i + 1) * P, 512:])
            for i in range(8):
                nc.gpsimd.dma_start(out=xt_sb[:, i, :1024],
                                    in_=xt[i * P:(i + 1) * P, :1024])
            for tt in range(8):
                nc.gpsimd.dma_start(out=xr_sb[:, tt, :],
                                    in_=xr[tt * P:(tt + 1) * P, :])
            for i in range(8):
                nc.gpsimd.dma_start(out=wv_sb[:, i, :],
                                    in_=wv[i * P:(i + 1) * P, :])
            for i in range(8):
                nc.gpsimd.dma_start(out=xt_sb[:, i, 1024:],
                                    in_=xt[i * P:(i + 1) * P, 1024:])
            for tt in range(8, 16):
                nc.gpsimd.dma_start(out=xr_sb[:, tt, :],
                                    in_=xr[tt * P:(tt + 1) * P, :])

            # ---------- Phase A/B interleaved schedule -------------------
            # G = (x_q M)^T + u, computed ct-outer over PSUM accumulators so
            # the first matmul needs only the first m/xtq DMA. The qc=0
            # column-half runs first on all 8 banks; then tiles 0-1 of the
            # attention phase run (they only need qc=0 scores + the first x
            # chunks) while the rest of the inputs stream in; then the qc=1
            # half runs on 2 banks; then the remaining tiles.
            #
            # Phase-B stages per tile t (nk = 256*(t+1), nkc = #512-chunks):
            #   S(t,c): fp8 DoubleRow score matmuls into psS
            #   E(t,c): mask (last chunk) + exp -> p_sb, rowsum accum, rinv
            #   TZ(t,kt): P^T transpose + copy + 2 Z matmuls into pz
            #   ZC(t): pz -> z_sb (bf16), normalized by 1/rowsum
            #   ZO(t): Z^T transposes + O matmuls (+ bv rank-1 fold) into po
            #   EP(t): po -> o_sb (scalar), DMA out
            # PE issue order pipelines: S of the next chunk covers exp; ZO of
            # the previous tile covers the z-copy; S(t,0..1) covers ZC(t-1).
            with tc.tile_pool(name="psA", bufs=1, space="PSUM") as psA:
                # warmup chain: keeps the PE p-state ramped across the
                # initial input-DMA wait so G streams at full clock
                for rep in range(6):
                    for dc in range(8):
                        pb = psA.tile([P, 512], F32, tag=f"g{dc}",
                                      name=f"pb{dc}")
                        nc.tensor.matmul(pb[:, :P], ones_sb, ones_sb,
                                         start=True, stop=True)
                        if rep == 5 and dc < 2:
                            nc.scalar.copy(warm_sb[:, dc * 2:dc * 2 + 2],
                                           pb[:, :2])

                for dth in range(2):
                    pg = [psA.tile([P, 512], F32, tag=f"g{dt}", name=f"pg{dt}")
                          for dt in range(dth * 4, dth * 4 + 4)]
                    for ct in range(8):
                        for di, dt in enumerate(range(dth * 4, dth * 4 + 4)):
                            nc.tensor.matmul(
                                pg[di],
                                m_sb[:, ct, dt * P:(dt + 1) * P],
                                xtq_sb[:, ct, :512],
                                start=(ct == 0),
                                stop=(ct == 7),
                            )
                    for di, dt in enumerate(range(dth * 4, dth * 4 + 4)):
                        # alternate engines so the copy chain halves in wall
                        # time (scalar activation vs DVE tensor-scalar add)
                        if di % 2 == 0:
                            nc.scalar.activation(
                                gt_sb[:, dt, :512], pg[di],
                                AF.Identity, bias=um_sb[:, dt:dt + 1])
                        else:
                            nc.vector.tensor_scalar_add(
                                gt_sb[:, dt, :512], pg[di],
                                um_sb[:, dt:dt + 1])

            with tc.tile_pool(name="bwork", bufs=2) as bwork, \
                 tc.tile_pool(name="psB", bufs=2, space="PSUM") as psS, \
                 tc.tile_pool(name="psPZ", bufs=1, space="PSUM") as psPZ:
                psT = psS   # transposes share the psB pool (tag ptp)

                state = {}   # per-tile buffers shared across stage fns

                def tile_head(t):
                    state[(t, "p")] = bwork.tile([P, 2048], BF16, tag="p",
                                                 name="p")
                    state[(t, "rsum")] = bwork.tile([P, 4], F32, tag="rsum",
                                                    name="rsum")
                    state[(t, "pz")] = [
                        psPZ.tile([P, 512], F32, tag=f"pz{dc}", name=f"pz{dc}")
                        for dc in range(2)]

                def S_stage(t, c, finalize=None):
                    nk = 256 * (t + 1)
                    nkc = (nk + 511) // 512
                    if finalize is None:
                        finalize = (c == nkc - 1)
                    w = min(512, nk - c * 512)
                    ps = psS.tile([P, 512], F32, tag="s", name="ps")
                    for i in range(4):
                        nc.tensor.matmul(
                            ps[:, :w],
                            gt_sb[:, 2 * i:2 * i + 2, t * P:(t + 1) * P],
                            xt_sb[:, 2 * i:2 * i + 2, c * 512:c * 512 + w],
                            start=(i == 0),
                            stop=(i == 3),
                            perf_mode=DR,
                        )
                    # E stage issues immediately after (scalar/vector queues)
                    if c == nkc - 1:
                        nc.vector.tensor_add(ps[:, w - 256:w],
                                             ps[:, w - 256:w], mask_sb)
                    nc.scalar.activation(
                        state[(t, "p")][:, c * 512:c * 512 + w], ps[:, :w],
                        AF.Exp, scale=SCALE,
                        accum_out=state[(t, "rsum")][:, c:c + 1])
                    if finalize:
                        # rowsum -> 1/rowsum right after the final exp so the
                        # z-copy can normalize Z (folds the softmax divide)
                        rtot = bwork.tile([P, 1], F32, tag="rtot", name="rtot")
                        rinv = bwork.tile([P, 1], F32, tag="rinv", name="rinv")
                        nc.vector.reduce_sum(rtot, state[(t, "rsum")][:, :nkc],
                                             axis=mybir.AxisListType.X)
                        nc.vector.reciprocal(rinv, rtot)
                        state[(t, "rinv")] = rinv

                def TZ_stage(t, c, stop_kt=None):
                    nk = 256 * (t + 1)
                    if stop_kt is None:
                        stop_kt = nk // P - 1
                    w = min(512, nk - c * 512)
                    p_sb = state[(t, "p")]
                    pz = state[(t, "pz")]
                    for kt in range(c * 4, c * 4 + w // P):
                        ptp = psT.tile([P, P], BF16, tag="ptp", name="ptp")
                        nc.tensor.transpose(
                            ptp, p_sb[:, kt * P:(kt + 1) * P], ident)
                        pt_sb = bwork.tile([P, P], BF16, tag="pt", name="pt_sb")
                        nc.vector.tensor_copy(pt_sb, ptp)
                        for dc in range(2):
                            nc.tensor.matmul(
                                pz[dc],
                                pt_sb,
                                xr_sb[:, kt, dc * 512:(dc + 1) * 512],
                                start=(kt == 0),
                                stop=(kt == stop_kt),
                            )

                def ZC_stage(t):
                    # pz -> z_sb normalized by 1/rowsum (scalar and vector in
                    # parallel, one 512-chunk each). Tag per t%4: up to four
                    # z tiles are alive while their ZO stages are deferred.
                    z_sb = bwork.tile([P, D], BF16, tag=f"z{t % 4}",
                                      name="z_sb")
                    state[(t, "z")] = z_sb
                    rinv = state[(t, "rinv")]
                    nc.scalar.activation(z_sb[:, :512], state[(t, "pz")][0],
                                         AF.Copy, scale=rinv)
                    nc.vector.tensor_scalar_mul(z_sb[:, 512:],
                                                state[(t, "pz")][1], rinv)

                # ---- tiles 0-3 early: scores + Z only (ZO deferred) ------
                # They only need the qc=0 half of G and the first x chunks,
                # so they fill the input-DMA window with real PE work. The
                # qc=1 half of G runs on two spare PSUM banks, its quarter
                # passes interleaved so they cover the z-copy latencies.
                def g_qc1_quarter(psA2, q4):
                    pg = [psA2.tile([P, 512], F32, tag=f"h{j}", name=f"ph{j}")
                          for j in range(2)]
                    for ct in range(8):
                        for j in range(2):
                            dt = q4 * 2 + j
                            nc.tensor.matmul(
                                pg[j],
                                m_sb[:, ct, dt * P:(dt + 1) * P],
                                xtq_sb[:, ct, 512:],
                                start=(ct == 0),
                                stop=(ct == 7),
                            )
                    for j in range(2):
                        dt = q4 * 2 + j
                        if j == 0:
                            nc.scalar.activation(
                                gt_sb[:, dt, 512:], pg[j],
                                AF.Identity, bias=um_sb[:, dt:dt + 1])
                        else:
                            nc.vector.tensor_scalar_add(
                                gt_sb[:, dt, 512:], pg[j],
                                um_sb[:, dt:dt + 1])

                tile_head(0)
                S_stage(0, 0)
                tile_head(1)
                S_stage(1, 0)
                TZ_stage(0, 0)
                ZC_stage(0)
                tile_head(2)
                S_stage(2, 0)
                S_stage(2, 1)
                TZ_stage(1, 0)
                ZC_stage(1)
                tile_head(3)
                S_stage(3, 0)
                S_stage(3, 1)
                TZ_stage(2, 0)
                TZ_stage(2, 1)
                ZC_stage(2)
                TZ_stage(3, 0)
                TZ_stage(3, 1)
                ZC_stage(3)
                with tc.tile_pool(name="psA2", bufs=1, space="PSUM") as psA2:
                    for q4 in range(4):
                        g_qc1_quarter(psA2, q4)
                    for dc in range(2):
                        pb = psA2.tile([P, 512], F32, tag=f"h{dc}",
                                       name=f"pbv{dc}")
                        nc.tensor.matmul(pb, ones_sb,
                                         bvr_sb[:, dc * 512:(dc + 1) * 512],
                                         start=True, stop=True)
                        nc.scalar.copy(bvb_sb[:, dc * 512:(dc + 1) * 512], pb)

                # ---- remaining tiles, with ZO/EP pipelined one tile back --
                with tc.tile_pool(name="psC", bufs=1, space="PSUM") as psC:

                    def ZO_stage(t):
                        z_sb = state[(t, "z")]
                        po = [psC.tile([P, 512], F32, tag=f"po{dc}",
                                       name=f"po{dc}")
                              for dc in range(2)]
                        state[(t, "po")] = po
                        for cc in range(8):
                            ztp = psT.tile([P, P], BF16, tag="ptp", name="ztp")
                            nc.tensor.transpose(
                                ztp, z_sb[:, cc * P:(cc + 1) * P], ident)
                            zt_sb = bwork.tile([P, P], BF16, tag="zt",
                                               name="zt_sb")
                            nc.vector.tensor_copy(zt_sb, ztp)
                            for dc in range(2):
                                nc.tensor.matmul(
                                    po[dc],
                                    zt_sb,
                                    wv_sb[:, cc, dc * 512:(dc + 1) * 512],
                                    start=(cc == 0),
                                    stop=(cc == 7),
                                )
                            if cc == 0 and t == NQT - 1:
                                # final tile: fold bv on the PE (K=1) so the
                                # exposed tail skips the gpsimd add
                                for dc in range(2):
                                    nc.tensor.matmul(
                                        po[dc], ones_sb,
                                        bvr_sb[:, dc * 512:(dc + 1) * 512],
                                        start=False, stop=False)

                    def EP_stage(t, final=False):
                        # po is the finished output (normalized, biased);
                        # stage through SBUF on scalar (keeps the vector
                        # queue free for the latency-critical pt copies) and
                        # DMA out. The last tile has an idle vector queue, so
                        # split across engines with per-half DMAs instead.
                        po = state[(t, "po")]
                        o_sb = bwork.tile([P, D], F32, tag="o", name="o_sb")
                        if final:
                            nc.scalar.copy(o_sb[:, :512], po[0])
                            nc.sync.dma_start(
                                out=out[t * P:(t + 1) * P, :512],
                                in_=o_sb[:, :512])
                            nc.vector.tensor_copy(o_sb[:, 512:], po[1])
                            nc.sync.dma_start(
                                out=out[t * P:(t + 1) * P, 512:],
                                in_=o_sb[:, 512:])
                        else:
                            for dc in range(2):
                                sl = slice(dc * 512, (dc + 1) * 512)
                                nc.scalar.copy(o_sb[:, sl], po[dc])
                                nc.gpsimd.tensor_add(o_sb[:, sl], o_sb[:, sl],
                                                     bvb_sb[:, sl])
                            nc.sync.dma_start(out=out[t * P:(t + 1) * P, :],
                                              in_=o_sb)

                    # Deferred ZO/EP stages drain two-per-tile while the
                    # remaining tiles' score/Z work keeps the PE fed.
                    pending = [0, 1, 2, 3]
                    ZO_stage(pending.pop(0))
                    EP_stage(0)
                    for t in range(4, NQT - 1):
                        nk = 256 * (t + 1)
                        nkc = (nk + 511) // 512
                        tile_head(t)
                        S_stage(t, 0)
                        S_stage(t, 1)
                        done_s = 2
                        zo = pending.pop(0)
                        ZO_stage(zo)
                        EP_stage(zo)
                        for c in range(nkc):
                            TZ_stage(t, c)
                            if done_s < nkc:
                                S_stage(t, done_s)
                                done_s += 1
                            if c == 1 and t <= 5 and pending:
                                zo = pending.pop(0)
                                ZO_stage(zo)
                                EP_stage(zo)
                        ZC_stage(t)
                        pending.append(t)
                    # tile 7 processes its masked chunk (3) third and the
                    # plain chunk 2 last, so the end-of-tile rowsum chain
                    # (mask -> exp -> reduce -> recip -> z-copy) is off the
                    # critical path; the Z accumulation stops at kt=11.
                    t = NQT - 1
                    tile_head(t)
                    S_stage(t, 0)
                    S_stage(t, 1)
                    zo = pending.pop(0)
                    ZO_stage(zo)
                    EP_stage(zo)
                    TZ_stage(t, 0, stop_kt=11)
                    S_stage(t, 3, finalize=False)
                    TZ_stage(t, 1, stop_kt=11)
                    S_stage(t, 2, finalize=True)
                    TZ_stage(t, 3, stop_kt=11)
                    TZ_stage(t, 2, stop_kt=11)
                    ZC_stage(t)
                    pending.append(t)
                    while pending:
                        zo = pending.pop(0)
                        ZO_stage(zo)
                        EP_stage(zo, final=not pending)

    nc.finalize()
    return nc


def _prep_inputs(x, wq, bq, wk, bk, wv, bv):
    bf = ml_dtypes.bfloat16
    f8 = ml_dtypes.float8_e4m3
    wq32 = np.asarray(wq, np.float32)
    wk32 = np.asarray(wk, np.float32)
    m_host = (wq32 @ wk32.T).astype(bf)                 # Wq Wk^T
    u_host = (wk32 @ np.asarray(bq, np.float32))        # Wk bq, [D]
    um = np.ascontiguousarray(u_host.reshape(8, P).T).astype(np.float32)
    wv_b = np.ascontiguousarray(wv, np.float32).astype(bf)
    bvr = np.asarray(bv, np.float32).reshape(1, D).astype(bf)

    i = np.arange(P)[:, None]
    j = np.arange(256)[None, :]
    masks = [np.where(j <= i + P * h, 0.0, NEG).astype(np.float32)
             for h in range(2)]

    in_maps = []
    for core in range(8):
        b, h = core // 2, core % 2
        xb = np.asarray(x[b], np.float32)
        xT = np.ascontiguousarray(xb.T)
        xR = xb.astype(bf)
        qcols = (np.arange(8)[:, None] * 2 + h) * P + np.arange(P)[None, :]
        xTq = np.ascontiguousarray(xT[:, qcols.ravel()]).astype(bf)
        in_maps.append({
            "xt": xT.astype(f8), "xr": xR, "xtq": xTq, "mm_w": m_host,
            "wv": wv_b, "um": um, "bvr": bvr, "mask": masks[h],
        })
    return in_maps


def kernel(x, wq, bq, wk, bk, wv, bv, _trace=False, _trace_kwargs=None):
    if "nc" not in _CACHED:
        _CACHED["nc"] = build_nc()
    nc = _CACHED["nc"]
    in_maps = _prep_inputs(x, wq, bq, wk, bk, wv, bv)
    kw = {}
    if _trace:
        kw = dict(trace=True, **(_trace_kwargs or {}))
    res = run_bass_kernel_spmd(nc, in_maps, list(range(8)), **kw)
    out = np.empty((B, L, D), np.float32)
    for core in range(8):
        b, h = core // 2, core % 2
        o = np.asarray(res.results[core]["out"], np.float32)
        out[b].reshape(16, P, D)[h::2] = o.reshape(NQT, P, D)
    if _trace:
        _CACHED["last_results"] = res
    return out


# revision 30
# speedup vs baseline: 1.0520x; 1.0258x over previous
"""MiniCausalAttention on 8 NeuronCores (Trainium2, Bass/Tile).

Problem: x[4,2048,1024] fp32; q/k/v = x@w+b; causal softmax(q k^T/sqrt(D)) @ v.

Sharding: 8 cores = (batch b in 0..3) x (half h in 0..1). Core (b,h) handles
query tiles t' = 2t+h for t in 0..7 (interleaved 128-row tiles), so every
core sees the SAME set of causal key-extents nk(t) = 256*(t+1) -> one SPMD
program, perfectly balanced.

Projection reassociation (exact algebra, host-precomputed M = Wq Wk^T and
u = Wk bq):
  scores  S = q k^T = x_q M x^T + 1 (x) (x u)^T  (+ per-query terms that
          cancel in softmax and are dropped)
  output  O = P_norm v = [(P x) Wv] / rowsum + bv
so neither K nor V is ever materialized. The key-bias u·x_k term is folded
into G^T = (x_q M)^T at PSUM->SBUF copy time as a per-partition activation
bias (G'^T[d,q] = G^T[d,q] + u[d]); 1/rowsum is folded into the Z copy; the
output bias bv is folded into the O accumulation as a rank-1 K=1 matmul.

Precision: G is computed in bf16 (scores accuracy), then stored fp8-e4m3;
x^T is shipped fp8 -> the S matmul runs fp8 DoubleRow (K=256/instruction,
1 col/cycle = 2x bf16 FLOP rate) costing ~1.3e-2 rel err total (tol 2e-2,
validated in numpy sim; fp8 anywhere in the P*V path fails tolerance, and
fp8 multi-plane residual tricks lose to bf16 since DR is 1 cyc/col).
P/Z/O stay bf16; PSUM accumulation fp32; softmax statistics fp32.

Scheduling (PE is the bottleneck; it streams 1 col/cycle at 2.0-2.4GHz with
LDWEIGHTS fully hidden, so only column count and stalls matter):
- Inputs stream in first-use order on two DMA queues; the m/xtq weights for
  the score projection G lead so the ct-outer G loop (8 PSUM accumulators,
  dt-halves with scalar/vector-alternated copies) starts ~1.5us after DMA.
- Tiles 0-3 of phase B run right after the qc=0 half of G (they only need
  the first x chunks), filling the input-DMA window with PE work; their
  ZO/EP stages are deferred. The qc=1 half of G then runs on 2 spare PSUM
  banks, and tiles 4-7 drain the deferred ZO stages two per tile.
- Within a tile the PE issue order software-pipelines:
    S(t,0) S(t,1) ZO(prev) EP(prev) TZ(t,0) S(t,2) TZ(t,1) ... ZC(t)
  so the exp latency (scalar), the z-copy, and the o-copy all hide under
  score/Z matmuls. Tile 7 processes its masked chunk third so the closing
  rowsum chain stays off the critical path, and its EP splits across the
  scalar+vector engines with per-half DMAs.
"""

import sys

if "/opt/trn_rl_repo" not in sys.path:
    sys.path.insert(0, "/opt/trn_rl_repo")

import numpy as np
import ml_dtypes

import concourse.bass as bass  # noqa: F401
import concourse.tile as tile
from concourse import bacc, mybir
from concourse.bass_utils import run_bass_kernel_spmd
from concourse.masks import make_identity

BF16 = mybir.dt.bfloat16
F32 = mybir.dt.float32
FP8 = mybir.dt.float8e4
AF = mybir.ActivationFunctionType
DR = mybir.MatmulPerfMode.DoubleRow

B, L, D = 4, 2048, 1024
P = 128
NQT = 8          # q-tiles per core, 128 rows each
SCALE = 1.0 / 32.0   # 1/sqrt(D)
NEG = -1.0e30

_CACHED = {}


def build_nc():
    nc = bacc.Bacc(None, target_bir_lowering=False)

    xt = nc.declare_dram_parameter("xt", [D, L], FP8, isOutput=False)      # x^T fp8
    xr = nc.declare_dram_parameter("xr", [L, D], BF16, isOutput=False)     # x rows
    xtq = nc.declare_dram_parameter("xtq", [D, D], BF16, isOutput=False)   # q cols of x^T
    mm_w = nc.declare_dram_parameter("mm_w", [D, D], BF16, isOutput=False)  # Wq Wk^T
    wv = nc.declare_dram_parameter("wv", [D, D], BF16, isOutput=False)
    um = nc.declare_dram_parameter("um", [P, 8], F32, isOutput=False)      # Wk bq
    bvr = nc.declare_dram_parameter("bvr", [1, D], BF16, isOutput=False)
    mask = nc.declare_dram_parameter("mask", [P, 256], F32, isOutput=False)
    out = nc.declare_dram_parameter("out", [D, D], F32, isOutput=True)

    with tile.TileContext(nc) as tc:
        with tc.tile_pool(name="persist", bufs=1) as persist:
            xt_sb = persist.tile([P, 8, L], FP8)     # x^T: [d-part, ct, token]
            xr_sb = persist.tile([P, 16, D], BF16)   # x: [tok-part, tt, d]
            gt_sb = persist.tile([P, 8, D], FP8)     # G'^T: [d-part, dt, qcol]
            xtq_sb = persist.tile([P, 8, D], BF16)
            m_sb = persist.tile([P, 8, D], BF16)
            wv_sb = persist.tile([P, 8, D], BF16)
            um_sb = persist.tile([P, 8], F32)
            bvr_sb = persist.tile([1, D], BF16)
            mask_sb = persist.tile([P, 256], F32)
            ident = persist.tile([P, P], BF16)
            ones_sb = persist.tile([1, P], BF16)
            warm_sb = persist.tile([P, 4], F32)      # warmup matmul sink
            bvb_sb = persist.tile([P, D], F32)       # broadcast bias 1 (x) bv

            make_identity(nc, ident)
            nc.vector.memset(ones_sb, 1.0)

            # Input streams, ordered by first use. The sync queue carries the
            # critical-path stream in exact need-order (grouped transfers to
            # stay bandwidth- not issue-bound): m/xtq-h1 for G(qc0), then the
            # x chunks tiles 0-3 consume, then xtq-h2 for G(qc1). The gpsimd
            # queue carries the late-needed bulk (wv for ZO, xt-c1/xr-hi for
            # tiles 4-7).
            for i in range(2):
                nc.sync.dma_start(out=m_sb[:, i, :],
                                  in_=mm_w[i * P:(i + 1) * P, :])
            for i in range(4):
                nc.sync.dma_start(out=xtq_sb[:, i, :512],
                                  in_=xtq[i * P:(i + 1) * P, :512])
            for i in range(2, 6):
                nc.sync.dma_start(out=m_sb[:, i, :],
                                  in_=mm_w[i * P:(i + 1) * P, :])
            for i in range(4, 8):
                nc.sync.dma_start(out=xtq_sb[:, i, :512],
                                  in_=xtq[i * P:(i + 1) * P, :512])
            for i in range(6, 8):
                nc.sync.dma_start(out=m_sb[:, i, :],
                                  in_=mm_w[i * P:(i + 1) * P, :])
            nc.sync.dma_start(out=um_sb, in_=um[:, :])
            nc.sync.dma_start(out=bvr_sb, in_=bvr[:, :])
            nc.sync.dma_start(out=mask_sb, in_=mask[:, :])
            for i in range(8):
                nc.sync.dma_start(out=xtq_sb[:, i, 512:],
                                  in_=xtq[i * P:(i + 1) * P, 512:])
            for i in range(8):
                nc.sync.dma_start(out=wv_sb[:, i, :],
                                  in_=wv[i * P:(i + 1) * P, :])
            for i in range(8):
                nc.gpsimd.dma_start(out=xt_sb[:, i, :1024],
                                    in_=xt[i * P:(i + 1) * P, :1024])
            for tt in range(8):
                nc.gpsimd.dma_start(out=xr_sb[:, tt, :],
                                    in_=xr[tt * P:(tt + 1) * P, :])
            for i in range(8):
                nc.gpsimd.dma_start(out=xt_sb[:, i, 1024:],
                                    in_=xt[i * P:(i + 1) * P, 1024:])
            for tt in range(8, 16):
                nc.gpsimd.dma_start(out=xr_sb[:, tt, :],
                                    in_=xr[tt * P:(tt + 1) * P, :])

            # ---------- Phase A/B interleaved schedule -------------------
            # G = (x_q M)^T + u, computed ct-outer over PSUM accumulators so
            # the first matmul needs only the first m/xtq DMA. The qc=0
            # column-half runs first on all 8 banks; then tiles 0-1 of the
            # attention phase run (they only need qc=0 scores + the first x
            # chunks) while the rest of the inputs stream in; then the qc=1
            # half runs on 2 banks; then the remaining tiles.
            #
            # Phase-B stages per tile t (nk = 256*(t+1), nkc = #512-chunks):
            #   S(t,c): fp8 DoubleRow score matmuls into psS
            #   E(t,c): mask (last chunk) + exp -> p_sb, rowsum accum, rinv
            #   TZ(t,kt): P^T transpose + copy + 2 Z matmuls into pz
            #   ZC(t): pz -> z_sb (bf16), normalized by 1/rowsum
            #   ZO(t): Z^T transposes + O matmuls (+ bv rank-1 fold) into po
            #   EP(t): po -> o_sb (scalar), DMA out
            # PE issue order pipelines: S of the next chunk covers exp; ZO of
            # the previous tile covers the z-copy; S(t,0..1) covers ZC(t-1).
            with tc.tile_pool(name="psA", bufs=1, space="PSUM") as psA:
                # warmup chain: keeps the PE p-state ramped across the
                # initial input-DMA wait so G streams at full clock
                for rep in range(6):
                    for dc in range(8):
                        pb = psA.tile([P, 512], F32, tag=f"g{dc}",
                                      name=f"pb{dc}")
                        nc.tensor.matmul(pb[:, :P], ones_sb, ones_sb,
                                         start=True, stop=True)
                        if rep == 5 and dc < 2:
                            nc.scalar.copy(warm_sb[:, dc * 2:dc * 2 + 2],
                                           pb[:, :2])

                for dth in range(2):
                    pg = [psA.tile([P, 512], F32, tag=f"g{dt}", name=f"pg{dt}")
                          for dt in range(dth * 4, dth * 4 + 4)]
                    for ct in range(8):
                        for di, dt in enumerate(range(dth * 4, dth * 4 + 4)):
                            nc.tensor.matmul(
                                pg[di],
                                m_sb[:, ct, dt * P:(dt + 1) * P],
                                xtq_sb[:, ct, :512],
                                start=(ct == 0),
                                stop=(ct == 7),
                            )
                    for di, dt in enumerate(range(dth * 4, dth * 4 + 4)):
                        # alternate engines so the copy chain halves in wall
                        # time (scalar activation vs DVE tensor-scalar add)
                        if di % 2 == 0:
                            nc.scalar.activation(
                                gt_sb[:, dt, :512], pg[di],
                                AF.Identity, bias=um_sb[:, dt:dt + 1])
                        else:
                            nc.vector.tensor_scalar_add(
                                gt_sb[:, dt, :512], pg[di],
                                um_sb[:, dt:dt + 1])

            with tc.tile_pool(name="bwork", bufs=2) as bwork, \
                 tc.tile_pool(name="psB", bufs=2, space="PSUM") as psS, \
                 tc.tile_pool(name="psPZ", bufs=1, space="PSUM") as psPZ:
                psT = psS   # transposes share the psB pool (tag ptp)

                state = {}   # per-tile buffers shared across stage fns

                def tile_head(t):
                    state[(t, "p")] = bwork.tile([P, 2048], BF16, tag="p",
                                                 name="p")
                    state[(t, "rsum")] = bwork.tile([P, 4], F32, tag="rsum",
                                                    name="rsum")
                    state[(t, "pz")] = [
                        psPZ.tile([P, 512], F32, tag=f"pz{dc}", name=f"pz{dc}")
                        for dc in range(2)]

                def S_stage(t, c, finalize=None):
                    nk = 256 * (t + 1)
                    nkc = (nk + 511) // 512
                    if finalize is None:
                        finalize = (c == nkc - 1)
                    w = min(512, nk - c * 512)
                    ps = psS.tile([P, 512], F32, tag="s", name="ps")
                    for i in range(4):
                        nc.tensor.matmul(
                            ps[:, :w],
                            gt_sb[:, 2 * i:2 * i + 2, t * P:(t + 1) * P],
                            xt_sb[:, 2 * i:2 * i + 2, c * 512:c * 512 + w],
                            start=(i == 0),
                            stop=(i == 3),
                            perf_mode=DR,
                        )
                    # E stage issues immediately after (scalar/vector queues)
                    if c == nkc - 1:
                        nc.vector.tensor_add(ps[:, w - 256:w],
                                             ps[:, w - 256:w], mask_sb)
                    nc.scalar.activation(
                        state[(t, "p")][:, c * 512:c * 512 + w], ps[:, :w],
                        AF.Exp, scale=SCALE,
                        accum_out=state[(t, "rsum")][:, c:c + 1])
                    if finalize:
                        # rowsum -> 1/rowsum right after the final exp so the
                        # z-copy can normalize Z (folds the softmax divide)
                        rtot = bwork.tile([P, 1], F32, tag="rtot", name="rtot")
                        rinv = bwork.tile([P, 1], F32, tag="rinv", name="rinv")
                        nc.vector.reduce_sum(rtot, state[(t, "rsum")][:, :nkc],
                                             axis=mybir.AxisListType.X)
                        nc.vector.reciprocal(rinv, rtot)
                        state[(t, "rinv")] = rinv

                def TZ_stage(t, c, stop_kt=None):
                    nk = 256 * (t + 1)
                    if stop_kt is None:
                        stop_kt = nk // P - 1
                    w = min(512, nk - c * 512)
                    p_sb = state[(t, "p")]
                    pz = state[(t, "pz")]
                    for kt in range(c * 4, c * 4 + w // P):
                        ptp = psT.tile([P, P], BF16, tag="ptp", name="ptp")
                        nc.tensor.transpose(
                            ptp, p_sb[:, kt * P:(kt + 1) * P], ident)
                        pt_sb = bwork.tile([P, P], BF16, tag="pt", name="pt_sb")
                        nc.vector.tensor_copy(pt_sb, ptp)
                        for dc in range(2):
                            nc.tensor.matmul(
                                pz[dc],
                                pt_sb,
                                xr_sb[:, kt, dc * 512:(dc + 1) * 512],
                                start=(kt == 0),
                                stop=(kt == stop_kt),
                            )

                def ZC_stage(t):
                    # pz -> z_sb normalized by 1/rowsum (scalar and vector in
                    # parallel, one 512-chunk each). Tag per t%4: up to four
                    # z tiles are alive while their ZO stages are deferred.
                    z_sb = bwork.tile([P, D], BF16, tag=f"z{t % 4}",
                                      name="z_sb")
                    state[(t, "z")] = z_sb
                    rinv = state[(t, "rinv")]
                    nc.scalar.activation(z_sb[:, :512], state[(t, "pz")][0],
                                         AF.Copy, scale=rinv)
                    nc.vector.tensor_scalar_mul(z_sb[:, 512:],
                                                state[(t, "pz")][1], rinv)

                # ---- tiles 0-3 early: scores + Z only (ZO deferred) ------
                # They only need the qc=0 half of G and the first x chunks,
                # so they fill the input-DMA window with real PE work. The
                # qc=1 half of G runs on two spare PSUM banks, its quarter
                # passes interleaved so they cover the z-copy latencies.
                def g_qc1_quarter(psA2, q4):
                    pg = [psA2.tile([P, 512], F32, tag=f"h{j}", name=f"ph{j}")
                          for j in range(2)]
                    for ct in range(8):
                        for j in range(2):
                            dt = q4 * 2 + j
                            nc.tensor.matmul(
                                pg[j],
                                m_sb[:, ct, dt * P:(dt + 1) * P],
                                xtq_sb[:, ct, 512:],
                                start=(ct == 0),
                                stop=(ct == 7),
                            )
                    for j in range(2):
                        dt = q4 * 2 + j
                        if j == 0:
                            nc.scalar.activation(
                                gt_sb[:, dt, 512:], pg[j],
                                AF.Identity, bias=um_sb[:, dt:dt + 1])
                        else:
                            nc.vector.tensor_scalar_add(
                                gt_sb[:, dt, 512:], pg[j],
                                um_sb[:, dt:dt + 1])

                tile_head(0)
                S_stage(0, 0)
                tile_head(1)
                S_stage(1, 0)
                TZ_stage(0, 0)
                ZC_stage(0)
                tile_head(2)
                S_stage(2, 0)
                S_stage(2, 1)
                TZ_stage(1, 0)
                ZC_stage(1)
                tile_head(3)
                S_stage(3, 0)
                S_stage(3, 1)
                TZ_stage(2, 0)
                TZ_stage(2, 1)
                ZC_stage(2)
                TZ_stage(3, 0)
                TZ_stage(3, 1)
                ZC_stage(3)
                with tc.tile_pool(name="psA2", bufs=1, space="PSUM") as psA2:
                    for q4 in range(4):
                        g_qc1_quarter(psA2, q4)
                    for dc in range(2):
                        pb = psA2.tile([P, 512], F32, tag=f"h{dc}",
                                       name=f"pbv{dc}")
                        nc.tensor.matmul(pb, ones_sb,
                                         bvr_sb[:, dc * 512:(dc + 1) * 512],
                                         start=True, stop=True)
                        nc.scalar.copy(bvb_sb[:, dc * 512:(dc + 1) * 512], pb)

                # ---- remaining tiles, with ZO/EP pipelined one tile back --
                with tc.tile_pool(name="psC", bufs=1, space="PSUM") as psC:

                    def ZO_stage(t):
                        z_sb = state[(t, "z")]
                        po = [psC.tile([P, 512], F32, tag=f"po{dc}",
                                       name=f"po{dc}")
                              for dc in range(2)]
                        state[(t, "po")] = po
                        for cc in range(8):
                            ztp = psT.tile([P, P], BF16, tag="ptp", name="ztp")
                            nc.tensor.transpose(
                                ztp, z_sb[:, cc * P:(cc + 1) * P], ident)
                            zt_sb = bwork.tile([P, P], BF16, tag="zt",
                                               name="zt_sb")
                            nc.vector.tensor_copy(zt_sb, ztp)
                            for dc in range(2):
                                nc.tensor.matmul(
                                    po[dc],
                                    zt_sb,
                                    wv_sb[:, cc, dc * 512:(dc + 1) * 512],
                                    start=(cc == 0),
                                    stop=(cc == 7),
                                )
                            if cc == 0 and t == NQT - 1:
                                # final tile: fold bv on the PE (K=1) so the
                                # exposed tail skips the gpsimd add
                                for dc in range(2):
                                    nc.tensor.matmul(
                                        po[dc], ones_sb,
                                        bvr_sb[:, dc * 512:(dc + 1) * 512],
                                        start=False, stop=False)

                    def EP_stage(t, final=False):
                        # po is the finished output (normalized, biased);
                        # stage through SBUF on scalar (keeps the vector
                        # queue free for the latency-critical pt copies) and
                        # DMA out. The last tile has an idle vector queue, so
                        # split across engines with per-half DMAs instead.
                        po = state[(t, "po")]
                        o_sb = bwork.tile([P, D], F32, tag="o", name="o_sb")
                        if final:
                            nc.scalar.copy(o_sb[:, :512], po[0])
                            nc.sync.dma_start(
                                out=out[t * P:(t + 1) * P, :512],
                                in_=o_sb[:, :512])
                            nc.vector.tensor_copy(o_sb[:, 512:], po[1])
                            nc.sync.dma_start(
                                out=out[t * P:(t + 1) * P, 512:],
                                in_=o_sb[:, 512:])
                        else:
                            for dc in range(2):
                                sl = slice(dc * 512, (dc + 1) * 512)
                                nc.scalar.copy(o_sb[:, sl], po[dc])
                                nc.gpsimd.tensor_add(o_sb[:, sl], o_sb[:, sl],
                                                     bvb_sb[:, sl])
                            nc.sync.dma_start(out=out[t * P:(t + 1) * P, :],
                                              in_=o_sb)

                    # Deferred ZO/EP stages drain two-per-tile while the
                    # remaining tiles' score/Z work keeps the PE fed.
                    pending = [0, 1, 2, 3]
                    ZO_stage(pending.pop(0))
                    EP_stage(0)
                    for t in range(4, NQT - 1):
                        nk = 256 * (t + 1)
                        nkc = (nk + 511) // 512
                        tile_head(t)
                        S_stage(t, 0)
                        S_stage(t, 1)
                        done_s = 2
                        zo = pending.pop(0)
                        ZO_stage(zo)
                        EP_stage(zo)
                        for c in range(nkc):
                            TZ_stage(t, c)
                            if done_s < nkc:
                                S_stage(t, done_s)
                                done_s += 1
                            if c == 1 and t <= 5 and pending:
                                zo = pending.pop(0)
                                ZO_stage(zo)
                                EP_stage(zo)
                        ZC_stage(t)
                        pending.append(t)
                    # tile 7 processes its masked chunk (3) third and the
                    # plain chunk 2 last, so the end-of-tile rowsum chain
                    # (mask -> exp -> reduce -> recip -> z-copy) is off the
                    # critical path; the Z accumulation stops at kt=11.
                    t = NQT - 1
                    tile_head(t)
                    S_stage(t, 0)
                    S_stage(t, 1)
                    zo = pending.pop(0)
                    ZO_stage(zo)
                    EP_stage(zo)
                    TZ_stage(t, 0, stop_kt=11)
                    S_stage(t, 3, finalize=False)
                    TZ_stage(t, 1, stop_kt=11)
                    S_stage(t, 2, finalize=True)
                    TZ_stage(t, 3, stop_kt=11)
                    TZ_stage(t, 2, stop_kt=11)
                    ZC_stage(t)
                    pending.append(t)
                    while pending:
                        zo = pending.pop(0)
                        ZO_stage(zo)
                        EP_stage(zo, final=not pending)

    nc.finalize()
    return nc


def _prep_inputs(x, wq, bq, wk, bk, wv, bv):
    bf = ml_dtypes.bfloat16
    f8 = ml_dtypes.float8_e4m3
    wq32 = np.asarray(wq, np.float32)
    wk32 = np.asarray(wk, np.float32)
    m_host = (wq32 @ wk32.T).astype(bf)                 # Wq Wk^T
    u_host = (wk32 @ np.asarray(bq, np.float32))        # Wk bq, [D]
    um = np.ascontiguousarray(u_host.reshape(8, P).T).astype(np.float32)
    wv_b = np.ascontiguousarray(wv, np.float32).astype(bf)
    bvr = np.asarray(bv, np.float32).reshape(1, D).astype(bf)

    i = np.arange(P)[:, None]
    j = np.arange(256)[None, :]
    masks = [np.where(j <= i + P * h, 0.0, NEG).astype(np.float32)
             for h in range(2)]

    in_maps = []
    for core in range(8):
        b, h = core // 2, core % 2
        xb = np.asarray(x[b], np.float32)
        xT = np.ascontiguousarray(xb.T)
        xR = xb.astype(bf)
        qcols = (np.arange(8)[:, None] * 2 + h) * P + np.arange(P)[None, :]
        xTq = np.ascontiguousarray(xT[:, qcols.ravel()]).astype(bf)
        in_maps.append({
            "xt": xT.astype(f8), "xr": xR, "xtq": xTq, "mm_w": m_host,
            "wv": wv_b, "um": um, "bvr": bvr, "mask": masks[h],
        })
    return in_maps


def kernel(x, wq, bq, wk, bk, wv, bv, _trace=False, _trace_kwargs=None):
    if "nc" not in _CACHED:
        _CACHED["nc"] = build_nc()
    nc = _CACHED["nc"]
    in_maps = _prep_inputs(x, wq, bq, wk, bk, wv, bv)
    kw = {}
    if _trace:
        kw = dict(trace=True, **(_trace_kwargs or {}))
    res = run_bass_kernel_spmd(nc, in_maps, list(range(8)), **kw)
    out = np.empty((B, L, D), np.float32)
    for core in range(8):
        b, h = core // 2, core % 2
        o = np.asarray(res.results[core]["out"], np.float32)
        out[b].reshape(16, P, D)[h::2] = o.reshape(NQT, P, D)
    if _trace:
        _CACHED["last_results"] = res
    return out
